# revision 12
# baseline (speedup 1.0000x reference)
"""MoE FFN (capacity-gated routing) on 8 Trainium2 NeuronCores.

Strategy
--------
Expert-parallel: 16 experts / 8 cores = 2 experts per core; host-side
routing/dispatch/combine (full-I/O contract makes the dispatch gather
the sharding step). Experts are sorted by load and paired
heavy-with-light across cores, so the SPMD program uses two static
token widths (WA, WB) = per-slot maxima over cores.

v3: the grouped FFN runs on fp8 (float8e4 = e4m3) with DoubleRow
matmuls (2 contraction rows per partition, 0.5 cycles/row -> 4x bf16
column throughput) while keeping ~bf16 accuracy via an error-split:
every operand is represented as hi + lo fp8 terms sharing one scale
(x, W1, W2 split on the host; h split on-device: ACT writes f32 h,
DVE casts hh=fp8(h) and hl=fp8(h-hh)). Each matmul accumulates the
three significant cross terms (hi*hi + hi*lo + lo*hi) into one PSUM
group; the dropped lo*lo term is O(3e-4) relative. Per-column weight
scales (folded with the global x scale) are applied as per-partition
ScalarE activation scales on PSUM readout, so dequantization is free.
Weight bytes stay 2/element (same DMA as bf16) but PE time drops 25%,
moving the kernel from PE-bound to the HBM roofline. Expert outputs
return as bf16.
"""

import sys

if "/opt/trn_rl_repo" not in sys.path:
    sys.path.append("/opt/trn_rl_repo")

import numpy as np
import ml_dtypes

import concourse.tile as tile
from concourse import bacc, mybir
from concourse.bass_utils import run_bass_kernel_spmd

# Problem shape (hardcoded per contract)
D = 512        # d_model
F = 2048       # d_ff
E = 16         # num experts
B = 2048       # max tokens
CAP = 320      # per-expert capacity = int(1.25 * ceil(B * 2 / E))
N_CORES = 8
EPC = E // N_CORES  # experts per core

P = 128
KT = D // P    # k-tiles over d_model (4)
KP = KT // 2   # DoubleRow k-pair instructions per term (2)
FT = F // P    # tiles over d_ff (16)
TP = FT // 2   # DoubleRow t-pair instructions per term in phase 2 (8)
CH1 = 8        # w1 chunks along f (2 f-tiles each)
CH2 = 8        # w2 chunks along t-pairs (1 t-pair each; the last chunk
               # then gates only 3 matmuls per output group in the tail)
M_PER = FT // CH1   # f-tiles per w1 chunk (2)
T_PER = TP // CH2   # t-pairs per w2 chunk (2)

SX = 16.0      # global x scale into fp8
WTGT = 224.0   # per-column weight scale target (e4m3 max 448/240; stay safe)

FP8 = mybir.dt.float8e4
BF16 = mybir.dt.bfloat16
F32 = mybir.dt.float32
NPFP8 = ml_dtypes.float8_e4m3
NPBF16 = ml_dtypes.bfloat16

DR = mybir.MatmulPerfMode.DoubleRow

_CACHE = {}


def _build_nc(rep=1, act=None, wa=CAP, wb=CAP, with_bias=True):
    """Per-core program: grouped FFN for 2 experts (widths wa >= wb).

    fp8 hi/lo DoubleRow pipeline; see module docstring. All tensors are
    d-major; weights/xg arrive pre-packed in per-partition byte order
    (see _prep_in_maps) so every DMA is [128 x contiguous].
    """
    if act is None:
        act = mybir.ActivationFunctionType.Gelu
    nc = bacc.Bacc(None)
    widths = (wa, wb)
    # xg{s}: [P, hl(2), kp(2), i(2), w] fp8; row (kp*256 + i*128 + p) of
    # scaled x, hi term then lo term.
    xg = [
        nc.declare_dram_parameter(f"xg{s}", [P, 8 * widths[s]], FP8, isOutput=False)
        for s in range(EPC)
    ]
    # w1[e, c]: [P, hl(2), kp(2), i(2), 256] fp8 covering f cols
    # [c*256, (c+1)*256) of scaled W1.
    w1 = nc.declare_dram_parameter(
        "w1", [EPC, CH1, P, 8 * M_PER * P], FP8, isOutput=False
    )
    # w2[e, c]: [P, hl(2), tpl(2), i(2), 512] fp8 covering contraction
    # rows [(2c)*256, (2c+2)*256) of scaled W2.
    w2 = nc.declare_dram_parameter(
        "w2", [EPC, CH2, P, 4 * T_PER * D], FP8, isOutput=False
    )
    # scs: per-partition dequant scales (+ biases when present):
    # [P, s*(2*(FT+KT)) + m]: m<FT: 1/(SX*s1); m<FT+KT: 1/s2;
    # then FT+KT bias entries (b1, b2) in the same order.
    NSC = 2 * (FT + KT)
    scs = nc.declare_dram_parameter("scs", [P, EPC * NSC], F32, isOutput=False)
    yt = [
        nc.declare_dram_parameter(f"yt{s}", [P, KT * widths[s]], BF16, isOutput=True)
    for s in range(EPC)
    ]

    with (
        tile.TileContext(nc) as tc,
        tc.tile_pool(name="consts", bufs=1) as consts,
        tc.tile_pool(name="xgp", bufs=2) as xgp,
        tc.tile_pool(name="w1p", bufs=2 * CH1 + 1) as w1p,
        tc.tile_pool(name="w2p", bufs=2 * CH2 + 1) as w2p,
        tc.tile_pool(name="hp", bufs=2) as hp,
        tc.tile_pool(name="hhp", bufs=2) as hhp,
        tc.tile_pool(name="hlp", bufs=2) as hlp,
        tc.tile_pool(name="yp", bufs=2) as yp,
        tc.tile_pool(name="ps1", bufs=4, space="PSUM") as ps1,
        tc.tile_pool(name="ps2", bufs=4, space="PSUM") as ps2,
    ):
        scs_sb = consts.tile([P, EPC, NSC], F32, name="scs_sb")
        scs_issued = False

        # PE warm-up: dummy matmuls on a zeroed tile keep the PE busy (and
        # the p-state ramp running) while the first xg/w1 chunks stream in.
        zt = consts.tile([P, max(P, wa)], BF16, name="warm_zt")
        nc.vector.memset(zt, 0.0)
        for _w in range(6):
            pw = ps1.tile([P, wa], F32, tag="ps", name="warm_ps")
            nc.tensor.matmul(pw, zt[:, :P], zt[:, :wa], start=True, stop=True)

        def issue_w1(e, c):
            w1c = w1p.tile([P, 2, KP, 2, M_PER * P], FP8, tag="w1c", name="w1c")
            nc.sync.dma_start(
                out=w1c,
                in_=w1[e, c].rearrange(
                    "p (hl kp i f) -> p hl kp i f", hl=2, kp=KP, i=2
                ),
            )
            return w1c

        def issue_xg(e):
            nb = widths[e]
            xg_sb = xgp.tile([P, 2, KP, 2, nb], FP8, name="xg_sb")
            nc.sync.dma_start(
                out=xg_sb,
                in_=xg[e].rearrange("p (hl kp i n) -> p hl kp i n", hl=2, kp=KP, i=2),
            )
            return xg_sb

        prefetched = {}
        deferred_yt = [None]

        for _ in range(rep):
            for e in range(EPC):
                nb = widths[e]
                if e in prefetched:
                    xg_sb, w1c0 = prefetched.pop(e)
                else:
                    # w1c0 first (largest prefix of the first psum group),
                    # then xg; both must land before the first matmul
                    w1c0 = issue_w1(e, 0)
                    xg_sb = issue_xg(e)
                w1cs = [w1c0]
                h_sb = hp.tile([P, FT, nb], F32, name="h_sb")
                hh_sb = hhp.tile([P, FT, nb], FP8, name="hh_sb")
                hl_sb = hlp.tile([P, FT, nb], FP8, name="hl_sb")

                # phase 1: h = gelu((x @ W1) * scale1); 3 DR terms per psum
                # group; w1 chunked along f so matmuls start after one
                # 256KB chunk lands. The scs DMA and the previous expert's
                # deferred yt store slot into the chunk stream so small
                # transfers never cluster at the head of the HWDGE queue.
                for c in range(CH1):
                    if c > 0:
                        w1cs.append(issue_w1(e, c))
                    if c == 3 and deferred_yt[0] is not None:
                        deferred_yt[0]()
                        deferred_yt[0] = None
                    w1c = w1cs[c]
                    for ml in range(M_PER):
                        m = c * M_PER + ml
                        ps = ps1.tile([P, nb], F32, name="ps")
                        first = True
                        for whl, xhl in ((0, 0), (1, 0), (0, 1)):
                            for kp in range(KP):
                                nc.tensor.matmul(
                                    ps,
                                    w1c[:, whl, kp, :, ml * P : (ml + 1) * P],
                                    xg_sb[:, xhl, kp, :, :],
                                    start=first,
                                    stop=(whl == 0 and xhl == 1 and kp == KP - 1),
                                    perf_mode=DR,
                                )
                                first = False
                        if not scs_issued:
                            # traced before the first ACT (deps follow trace
                            # order) but issued via the Pool-engine SWDGE
                            # queue so it doesn't take an HWDGE slot from
                            # the weight-chunk stream
                            nc.gpsimd.dma_start(
                                out=scs_sb,
                                in_=scs.rearrange("p (e t) -> p e t", e=EPC),
                            )
                            scs_issued = True
                        nc.scalar.activation(
                            h_sb[:, m, :],
                            ps,
                            act,
                            bias=(
                                scs_sb[:, e, FT + KT + m : FT + KT + m + 1]
                                if with_bias
                                else 0.0
                            ),
                            scale=scs_sb[:, e, m : m + 1],
                        )
                        nc.vector.tensor_copy(out=hh_sb[:, m, :], in_=h_sb[:, m, :])
                        nc.vector.tensor_sub(
                            hl_sb[:, m, :], h_sb[:, m, :], hh_sb[:, m, :]
                        )

                # phase 2: y = (h @ W2) * scale2; tp-outer so each w2 chunk
                # is consumed on arrival; 4 output groups accumulate in
                # parallel across the 4 ps2 banks
                ps_y = [
                    ps2.tile([P, nb], F32, tag="psy", name=f"psy{m2}")
                    for m2 in range(KT)
                ]
                w2cs = []
                for c in range(CH2):
                    w2c = w2p.tile([P, 2, T_PER, 2, D], FP8, tag="w2c", name="w2c")
                    nc.sync.dma_start(
                        out=w2c,
                        in_=w2[e, c].rearrange(
                            "p (hl tpl i d) -> p hl tpl i d", hl=2, tpl=T_PER, i=2
                        ),
                    )
                    w2cs.append(w2c)
                    if c == 3 and e + 1 < EPC:
                        # prefetch the next expert's first-matmul inputs so
                        # its phase 1 starts as soon as this phase 2 ends
                        nw1c0 = issue_w1(e + 1, 0)
                        nxg = issue_xg(e + 1)
                        prefetched[e + 1] = (nxg, nw1c0)

                def mm2(tp, m2):
                    c, tpl = divmod(tp, T_PER)
                    first = True
                    for whl, hsrc in ((0, hh_sb), (1, hh_sb), (0, hl_sb)):
                        nc.tensor.matmul(
                            ps_y[m2],
                            w2cs[c][:, whl, tpl, :, m2 * P : (m2 + 1) * P],
                            hsrc[:, 2 * tp : 2 * tp + 2, :],
                            start=(tp == 0 and first),
                            stop=(tp == TP - 1 and whl == 0 and hsrc is hl_sb),
                            perf_mode=DR,
                        )
                        first = False

                y_sb = yp.tile([P, KT, nb], BF16, name="y_sb")

                def act_y(m2):
                    nc.scalar.activation(
                        y_sb[:, m2, :],
                        ps_y[m2],
                        mybir.ActivationFunctionType.Identity,
                        bias=(
                            scs_sb[
                                :, e, FT + KT + FT + m2 : FT + KT + FT + m2 + 1
                            ]
                            if with_bias
                            else 0.0
                        ),
                        scale=scs_sb[:, e, FT + m2 : FT + m2 + 1],
                    )

                last = e == EPC - 1
                yt_v = yt[e].rearrange("p (t n) -> p t n", t=KT)
                for tp in range(TP - 1 if last else TP):
                    for m2 in range(KT):
                        mm2(tp, m2)
                if not last:
                    for m2 in range(KT):
                        act_y(m2)

                    def defer(yv=yt_v, ys=y_sb):
                        nc.sync.dma_start(out=yv, in_=ys)

                    deferred_yt[0] = defer
                else:
                    # retire each group on only its 3 last-t-pair matmuls:
                    # the final w2 chunk (the last byte off HBM) gates just
                    # 12 matmuls total, and the acts/yt DMAs stagger behind
                    # it group by group (pieces >=992B/partition, full-rate)
                    for m2 in range(KT):
                        mm2(TP - 1, m2)
                        act_y(m2)
                        if m2 == 1:
                            nc.sync.dma_start(
                                out=yt_v[:, 0:2, :], in_=y_sb[:, 0:2, :]
                            )
                        elif m2 == 3:
                            nc.sync.dma_start(
                                out=yt_v[:, 2:4, :], in_=y_sb[:, 2:4, :]
                            )
    nc.finalize()
    return nc


def get_nc(rep=1, act=None, wa=CAP, wb=CAP, with_bias=True):
    key = (rep, act, wa, wb, with_bias)
    if key not in _CACHE:
        _CACHE[key] = _build_nc(rep, act, wa, wb, with_bias)
    return _CACHE[key]


def _route_np(routes):
    """Numpy replica of the reference's capacity-gated routing."""
    e_map = (routes.astype(np.int64) * E) // B                  # [B, K]
    sel0 = np.zeros((B, E), bool)
    np.put_along_axis(sel0, e_map, True, axis=1)
    sel0_i = sel0.astype(np.int32)
    cum = np.cumsum(sel0_i, axis=0) - sel0_i                    # exclusive cumsum
    selected = sel0 & (cum < CAP)
    slot = cum
    used = selected.sum(axis=1)
    tok_of_slot = np.zeros(E * CAP, np.int32)
    valid = np.zeros(E * CAP, bool)
    b_idx, e_idx = np.nonzero(selected)
    flat = e_idx * CAP + slot[b_idx, e_idx]
    tok_of_slot[flat] = b_idx
    valid[flat] = True
    return tok_of_slot, valid, used, selected, slot


def _assign(routing):
    """Pair experts heavy-with-light across cores.

    Returns (order, WA, WB): order[c] = (expert_A, expert_B) for core c,
    widths WA >= WB are maxima over cores, rounded up to 8.
    """
    selected = routing[3]
    loads = selected.sum(axis=0)
    s = np.argsort(-loads, kind="stable")
    order = [(int(s[i]), int(s[E - 1 - i])) for i in range(N_CORES)]
    wa = max(int(loads[a]) for a, _ in order)
    wb = max(int(loads[b]) for _, b in order)
    wa = min(CAP, max(32, -(-wa // 8) * 8))
    wb = min(CAP, max(32, -(-wb // 8) * 8))
    return order, wa, wb


def _fp8(v):
    return v.astype(NPFP8)


def _hilo(vs):
    """Same-scale hi/lo fp8 split of (already scaled) values."""
    hi = _fp8(vs)
    lo = _fp8(vs - hi.astype(np.float32))
    return hi, lo


def _col_scale(W):
    return WTGT / np.maximum(np.abs(W).max(axis=0), 1e-30)


def _pack_w1(W1e, s1):
    # scaled W1 [D, F] -> [CH1, P, hl*kp*i*256]; row kp*256+i*128+p,
    # f block c*256+fb
    Ws = W1e * s1[None, :]
    hi, lo = _hilo(Ws)
    # [D, F] index: [kp(2), i(2), p(128), c(8), fb(256)]
    def arr(v):
        return v.reshape(KP, 2, P, CH1, M_PER * P).transpose(3, 2, 0, 1, 4)
    a = np.stack([arr(hi), arr(lo)], axis=2)  # [c, p, hl, kp, i, fb]
    return np.ascontiguousarray(a.reshape(CH1, P, -1))


def _pack_w2(W2e, s2):
    # scaled W2 [F, D] -> [CH2, P, hl*tpl*i*512]; row (2c+tpl)*256+i*128+p
    Ws = W2e * s2[None, :]
    hi, lo = _hilo(Ws)
    # [F, D] index: [c(4), tpl(2), i(2), p(128), d(512)]
    def arr(v):
        return v.reshape(CH2, T_PER, 2, P, D).transpose(0, 3, 1, 2, 4)
    a = np.stack([arr(hi), arr(lo)], axis=2)  # [c, p, hl, tpl, i, d]
    return np.ascontiguousarray(a.reshape(CH2, P, -1))


def _pack_xg(x, routing, expert, width):
    """Gather expert tokens -> [P, hl*kp*i*width] fp8 (scaled by SX)."""
    tok_of_slot, valid, used, selected, slot = routing
    sl = slice(expert * CAP, expert * CAP + width)
    xgE = x[tok_of_slot[sl]] * valid[sl, None].astype(np.float32)   # [w, D]
    xs = xgE.T * SX                                                 # [D, w]
    hi, lo = _hilo(xs)
    # [D, w] -> [kp, i, p, w] -> [p, hl, kp, i, w]
    def arr(v):
        return v.reshape(KP, 2, P, width)
    a = np.stack([arr(hi), arr(lo)], axis=1)    # [kp, hl, i?] careful below
    # arr gives [kp, i, p, w]; stack axis=1 -> [kp, hl, i, p, w]
    a = a.transpose(3, 1, 0, 2, 4)              # [p, hl, kp, i, w]
    return np.ascontiguousarray(a.reshape(P, -1))


def _prep_in_maps(x, W1, b1, W2, b2, routing, order, wa, wb, with_bias=True):
    widths = (wa, wb)
    NSC = 2 * (FT + KT)
    in_maps = []
    s1s = [_col_scale(W1[e]) for e in range(E)]
    s2s = [_col_scale(W2[e]) for e in range(E)]
    w1p = {e: _pack_w1(W1[e], s1s[e]) for e in range(E)}
    w2p = {e: _pack_w2(W2[e], s2s[e]) for e in range(E)}
    for c in range(N_CORES):
        es = order[c]
        m = {
            "w1": np.stack([w1p[e] for e in es]),
            "w2": np.stack([w2p[e] for e in es]),
        }
        for s, e in enumerate(es):
            m[f"xg{s}"] = _pack_xg(x, routing, e, widths[s])
        scs = np.zeros((P, EPC, NSC), np.float32)
        for s, e in enumerate(es):
            scs[:, s, :FT] = (1.0 / (SX * s1s[e])).reshape(FT, P).T
            scs[:, s, FT : FT + KT] = (1.0 / s2s[e]).reshape(KT, P).T
            if with_bias:
                scs[:, s, FT + KT : FT + KT + FT] = b1[e].reshape(FT, P).T
                scs[:, s, FT + KT + FT :] = b2[e].reshape(KT, P).T
        m["scs"] = np.ascontiguousarray(scs.reshape(P, -1))
        in_maps.append(m)
    return in_maps


def _erf(v):
    # Abramowitz & Stegun 7.1.26, |err| <= 1.5e-7
    s = np.sign(v)
    a = np.abs(v)
    t = 1.0 / (1.0 + 0.3275911 * a)
    poly = t * (
        0.254829592
        + t * (-0.284496736 + t * (1.421413741 + t * (-1.453152027 + t * 1.061405429)))
    )
    return s * (1.0 - poly * np.exp(-a * a))


def _gelu_exact(v):
    return 0.5 * v * (1.0 + _erf(v / np.sqrt(2.0)))


def kernel(x, W1, b1, W2, b2, Wf1, bf1, Wf2, bf2, routes):
    x = np.asarray(x, np.float32)
    W1 = np.asarray(W1, np.float32)
    b1 = np.asarray(b1, np.float32)
    W2 = np.asarray(W2, np.float32)
    b2 = np.asarray(b2, np.float32)
    Wf1 = np.asarray(Wf1, np.float32)
    bf1 = np.asarray(bf1, np.float32)
    Wf2 = np.asarray(Wf2, np.float32)
    bf2 = np.asarray(bf2, np.float32)
    routes = np.asarray(routes)

    routing = _route_np(routes)
    tok_of_slot, valid, used, selected, slot = routing
    order, wa, wb = _assign(routing)
    with_bias = bool(np.any(b1) or np.any(b2))
    in_maps = _prep_in_maps(x, W1, b1, W2, b2, routing, order, wa, wb, with_bias)

    nc = get_nc(wa=wa, wb=wb, with_bias=with_bias)
    res = run_bass_kernel_spmd(nc, in_maps, core_ids=list(range(N_CORES)))

    # Per-expert outputs [D, width] f32 (garbage in invalid slots; never
    # read there).
    widths = (wa, wb)
    exp_out = [None] * E
    for c in range(N_CORES):
        for s, e in enumerate(order[c]):
            Yc = res.results[c][f"yt{s}"].astype(np.float32)
            exp_out[e] = Yc.reshape(P, KT, widths[s]).transpose(1, 0, 2).reshape(
                D, widths[s]
            )

    # Combine: each token was selected by <= 2 experts; gather its slot
    # outputs and average. Pure host-side gather.
    out = np.zeros((B, D), np.float32)
    b_idx, e_idx = np.nonzero(selected)                         # ordered by token
    s_of = slot[b_idx, e_idx]
    for e in range(E):
        msk = e_idx == e
        out[b_idx[msk]] += exp_out[e][:, s_of[msk]].T
    inv = (1.0 / np.maximum(used, 1)).astype(np.float32)
    out *= inv[:, None]

    # Overflow tokens (used == 0): exact fallback FFN on host.
    ovf = np.nonzero(used == 0)[0]
    if ovf.size:
        xo = x[ovf]
        fb = _gelu_exact(xo @ Wf1 + bf1) @ Wf2 + bf2
        out[ovf] = fb.astype(np.float32)

    return out.astype(np.float32)


# revision 14
# speedup vs baseline: 1.0160x; 1.0160x over previous
"""MoE FFN (capacity-gated routing) on 8 Trainium2 NeuronCores.

Strategy
--------
Expert-parallel: 16 experts / 8 cores = 2 experts per core; host-side
routing/dispatch/combine (full-I/O contract makes the dispatch gather
the sharding step). Experts are sorted by load and paired
heavy-with-light across cores, so the SPMD program uses two static
token widths (WA, WB) = per-slot maxima over cores.

v3: the grouped FFN runs on fp8 (float8e4 = e4m3) with DoubleRow
matmuls (2 contraction rows per partition, 0.5 cycles/row -> 4x bf16
column throughput) while keeping ~bf16 accuracy via an error-split:
every operand is represented as hi + lo fp8 terms sharing one scale
(x, W1, W2 split on the host; h split on-device: ACT writes f32 h,
DVE casts hh=fp8(h) and hl=fp8(h-hh)). Each matmul accumulates the
three significant cross terms (hi*hi + hi*lo + lo*hi) into one PSUM
group; the dropped lo*lo term is O(3e-4) relative. Per-column weight
scales (folded with the global x scale) are applied as per-partition
ScalarE activation scales on PSUM readout, so dequantization is free.
Weight bytes stay 2/element (same DMA as bf16) but PE time drops 25%,
moving the kernel from PE-bound to the HBM roofline. Expert outputs
return as bf16.
"""

import sys

if "/opt/trn_rl_repo" not in sys.path:
    sys.path.append("/opt/trn_rl_repo")

import numpy as np
import ml_dtypes

import concourse.tile as tile
from concourse import bacc, mybir
from concourse.bass_utils import run_bass_kernel_spmd

# Problem shape (hardcoded per contract)
D = 512        # d_model
F = 2048       # d_ff
E = 16         # num experts
B = 2048       # max tokens
CAP = 320      # per-expert capacity = int(1.25 * ceil(B * 2 / E))
N_CORES = 8
EPC = E // N_CORES  # experts per core

P = 128
KT = D // P    # k-tiles over d_model (4)
KP = KT // 2   # DoubleRow k-pair instructions per term (2)
FT = F // P    # tiles over d_ff (16)
TP = FT // 2   # DoubleRow t-pair instructions per term in phase 2 (8)
CH1 = 8        # w1 chunks along f (2 f-tiles each)
CH2 = 8        # w2 chunks along t-pairs (1 t-pair each; the last chunk
               # then gates only 3 matmuls per output group in the tail)
M_PER = FT // CH1   # f-tiles per w1 chunk (2)
T_PER = TP // CH2   # t-pairs per w2 chunk (2)

SX = 16.0      # global x scale into fp8
WTGT = 224.0   # per-column weight scale target (e4m3 max 448/240; stay safe)

FP8 = mybir.dt.float8e4
BF16 = mybir.dt.bfloat16
F32 = mybir.dt.float32
NPFP8 = ml_dtypes.float8_e4m3
NPBF16 = ml_dtypes.bfloat16

DR = mybir.MatmulPerfMode.DoubleRow

_CACHE = {}


def _build_nc(rep=1, act=None, wa=CAP, wb=CAP, with_bias=True):
    """Per-core program: grouped FFN for 2 experts (widths wa >= wb).

    fp8 hi/lo DoubleRow pipeline; see module docstring. All tensors are
    d-major; weights/xg arrive pre-packed in per-partition byte order
    (see _prep_in_maps) so every DMA is [128 x contiguous].
    """
    if act is None:
        act = mybir.ActivationFunctionType.Gelu
    nc = bacc.Bacc(None)
    widths = (wa, wb)
    # xg{s}: [P, hl(2), kp(2), i(2), w] fp8; row (kp*256 + i*128 + p) of
    # scaled x, hi term then lo term.
    xg = [
        nc.declare_dram_parameter(f"xg{s}", [P, 8 * widths[s]], FP8, isOutput=False)
        for s in range(EPC)
    ]
    # w1[e, c]: [P, hl(2), kp(2), i(2), 256] fp8 covering f cols
    # [c*256, (c+1)*256) of scaled W1.
    w1 = nc.declare_dram_parameter(
        "w1", [EPC, CH1, P, 8 * M_PER * P], FP8, isOutput=False
    )
    # w2[e, c]: [P, hl(2), tpl(2), i(2), 512] fp8 covering contraction
    # rows [(2c)*256, (2c+2)*256) of scaled W2.
    w2 = nc.declare_dram_parameter(
        "w2", [EPC, CH2, P, 4 * T_PER * D], FP8, isOutput=False
    )
    # scs: per-partition dequant scales (+ biases when present):
    # [P, s*(2*(FT+KT)) + m]: m<FT: 1/(SX*s1); m<FT+KT: 1/s2;
    # then FT+KT bias entries (b1, b2) in the same order.
    NSC = 2 * (FT + KT)
    scs = nc.declare_dram_parameter("scs", [P, EPC * NSC], F32, isOutput=False)
    yt = [
        nc.declare_dram_parameter(f"yt{s}", [P, KT * widths[s]], BF16, isOutput=True)
    for s in range(EPC)
    ]

    with (
        tile.TileContext(nc) as tc,
        tc.tile_pool(name="consts", bufs=1) as consts,
        tc.tile_pool(name="xgp", bufs=2) as xgp,
        tc.tile_pool(name="w1p", bufs=2 * CH1 + 1) as w1p,
        tc.tile_pool(name="w2p", bufs=2 * CH2 + 1) as w2p,
        tc.tile_pool(name="hp", bufs=2) as hp,
        tc.tile_pool(name="hhp", bufs=2) as hhp,
        tc.tile_pool(name="hlp", bufs=2) as hlp,
        tc.tile_pool(name="yp", bufs=2) as yp,
        tc.tile_pool(name="ps1", bufs=4, space="PSUM") as ps1,
        tc.tile_pool(name="ps2", bufs=4, space="PSUM") as ps2,
    ):
        scs_sb = consts.tile([P, EPC, NSC], F32, name="scs_sb")
        scs_issued = False

        # PE warm-up: dummy matmuls on a zeroed tile keep the PE busy (and
        # the p-state ramp running) while the first xg/w1 chunks stream in.
        zt = consts.tile([P, max(P, wa)], BF16, name="warm_zt")
        nc.vector.memset(zt, 0.0)
        for _w in range(6):
            pw = ps1.tile([P, wa], F32, tag="ps", name="warm_ps")
            nc.tensor.matmul(pw, zt[:, :P], zt[:, :wa], start=True, stop=True)

        def issue_w1(e, c):
            w1c = w1p.tile([P, 2, KP, 2, M_PER * P], FP8, tag="w1c", name="w1c")
            nc.sync.dma_start(
                out=w1c,
                in_=w1[e, c].rearrange(
                    "p (hl kp i f) -> p hl kp i f", hl=2, kp=KP, i=2
                ),
            )
            return w1c

        def issue_xg(e):
            nb = widths[e]
            xg_sb = xgp.tile([P, 2, KP, 2, nb], FP8, name="xg_sb")
            nc.sync.dma_start(
                out=xg_sb,
                in_=xg[e].rearrange("p (hl kp i n) -> p hl kp i n", hl=2, kp=KP, i=2),
            )
            return xg_sb

        prefetched = {}
        deferred_yt = [None]

        for _ in range(rep):
            for e in range(EPC):
                nb = widths[e]
                if e in prefetched:
                    xg_sb, w1c0 = prefetched.pop(e)
                else:
                    # w1c0 first (largest prefix of the first psum group),
                    # then xg; both must land before the first matmul
                    w1c0 = issue_w1(e, 0)
                    xg_sb = issue_xg(e)
                w1cs = [w1c0]
                h_sb = hp.tile([P, FT, nb], F32, name="h_sb")
                hh_sb = hhp.tile([P, FT, nb], FP8, name="hh_sb")
                hl_sb = hlp.tile([P, FT, nb], FP8, name="hl_sb")

                # phase 1: h = gelu((x @ W1) * scale1); 3 DR terms per psum
                # group; w1 chunked along f so matmuls start after one
                # 256KB chunk lands. The scs DMA and the previous expert's
                # deferred yt store slot into the chunk stream so small
                # transfers never cluster at the head of the HWDGE queue.
                for c in range(CH1):
                    if c > 0:
                        w1cs.append(issue_w1(e, c))
                    if c == 3 and deferred_yt[0] is not None:
                        deferred_yt[0]()
                        deferred_yt[0] = None
                    w1c = w1cs[c]
                    for ml in range(M_PER):
                        m = c * M_PER + ml
                        ps = ps1.tile([P, nb], F32, name="ps")
                        first = True
                        for whl, xhl in ((0, 0), (1, 0), (0, 1)):
                            for kp in range(KP):
                                nc.tensor.matmul(
                                    ps,
                                    w1c[:, whl, kp, :, ml * P : (ml + 1) * P],
                                    xg_sb[:, xhl, kp, :, :],
                                    start=first,
                                    stop=(whl == 0 and xhl == 1 and kp == KP - 1),
                                    perf_mode=DR,
                                )
                                first = False
                        if not scs_issued:
                            # traced before the first ACT (deps follow trace
                            # order) but issued via the Pool-engine SWDGE
                            # queue so it doesn't take an HWDGE slot from
                            # the weight-chunk stream
                            nc.gpsimd.dma_start(
                                out=scs_sb,
                                in_=scs.rearrange("p (e t) -> p e t", e=EPC),
                            )
                            scs_issued = True
                        nc.scalar.activation(
                            h_sb[:, m, :],
                            ps,
                            act,
                            bias=(
                                scs_sb[:, e, FT + KT + m : FT + KT + m + 1]
                                if with_bias
                                else 0.0
                            ),
                            scale=scs_sb[:, e, m : m + 1],
                        )
                        nc.vector.tensor_copy(out=hh_sb[:, m, :], in_=h_sb[:, m, :])
                        nc.vector.tensor_sub(
                            hl_sb[:, m, :], h_sb[:, m, :], hh_sb[:, m, :]
                        )

                # phase 2: y = (h @ W2) * scale2; tp-outer so each w2 chunk
                # is consumed on arrival; 4 output groups accumulate in
                # parallel across the 4 ps2 banks
                ps_y = [
                    ps2.tile([P, nb], F32, tag="psy", name=f"psy{m2}")
                    for m2 in range(KT)
                ]
                w2cs = []
                for c in range(CH2):
                    w2c = w2p.tile([P, 2, T_PER, 2, D], FP8, tag="w2c", name="w2c")
                    nc.sync.dma_start(
                        out=w2c,
                        in_=w2[e, c].rearrange(
                            "p (hl tpl i d) -> p hl tpl i d", hl=2, tpl=T_PER, i=2
                        ),
                    )
                    w2cs.append(w2c)
                    if c == 3 and e + 1 < EPC:
                        # prefetch the next expert's first-matmul inputs so
                        # its phase 1 starts as soon as this phase 2 ends
                        nw1c0 = issue_w1(e + 1, 0)
                        nxg = issue_xg(e + 1)
                        prefetched[e + 1] = (nxg, nw1c0)

                def mm2(tp, m2):
                    c, tpl = divmod(tp, T_PER)
                    first = True
                    for whl, hsrc in ((0, hh_sb), (1, hh_sb), (0, hl_sb)):
                        nc.tensor.matmul(
                            ps_y[m2],
                            w2cs[c][:, whl, tpl, :, m2 * P : (m2 + 1) * P],
                            hsrc[:, 2 * tp : 2 * tp + 2, :],
                            start=(tp == 0 and first),
                            stop=(tp == TP - 1 and whl == 0 and hsrc is hl_sb),
                            perf_mode=DR,
                        )
                        first = False

                y_sb = yp.tile([P, KT, nb], BF16, name="y_sb")

                def act_y(m2, on_dve=False):
                    if on_dve and not with_bias:
                        # spread the final dequant readouts across ACT and
                        # DVE so the last groups retire in parallel chains
                        nc.vector.tensor_scalar_mul(
                            y_sb[:, m2, :],
                            ps_y[m2],
                            scs_sb[:, e, FT + m2 : FT + m2 + 1],
                        )
                        return
                    nc.scalar.activation(
                        y_sb[:, m2, :],
                        ps_y[m2],
                        mybir.ActivationFunctionType.Identity,
                        bias=(
                            scs_sb[
                                :, e, FT + KT + FT + m2 : FT + KT + FT + m2 + 1
                            ]
                            if with_bias
                            else 0.0
                        ),
                        scale=scs_sb[:, e, FT + m2 : FT + m2 + 1],
                    )

                last = e == EPC - 1
                yt_v = yt[e].rearrange("p (t n) -> p t n", t=KT)
                for tp in range(TP - 1 if last else TP):
                    for m2 in range(KT):
                        mm2(tp, m2)
                if not last:
                    for m2 in range(KT):
                        act_y(m2)

                    def defer(yv=yt_v, ys=y_sb):
                        nc.sync.dma_start(out=yv, in_=ys)

                    deferred_yt[0] = defer
                else:
                    # retire each group on only its 3 last-t-pair matmuls:
                    # the final w2 chunk (the last byte off HBM) gates just
                    # 12 matmuls total, and the acts/yt DMAs stagger behind
                    # it group by group (pieces >=992B/partition, full-rate)
                    for m2 in range(KT):
                        mm2(TP - 1, m2)
                        act_y(m2, on_dve=(m2 % 2 == 1))
                        if m2 == 1:
                            nc.sync.dma_start(
                                out=yt_v[:, 0:2, :], in_=y_sb[:, 0:2, :]
                            )
                        elif m2 == 3:
                            nc.sync.dma_start(
                                out=yt_v[:, 2:4, :], in_=y_sb[:, 2:4, :]
                            )
    nc.finalize()
    return nc


def get_nc(rep=1, act=None, wa=CAP, wb=CAP, with_bias=True):
    key = (rep, act, wa, wb, with_bias)
    if key not in _CACHE:
        _CACHE[key] = _build_nc(rep, act, wa, wb, with_bias)
    return _CACHE[key]


def _route_np(routes):
    """Numpy replica of the reference's capacity-gated routing."""
    e_map = (routes.astype(np.int64) * E) // B                  # [B, K]
    sel0 = np.zeros((B, E), bool)
    np.put_along_axis(sel0, e_map, True, axis=1)
    sel0_i = sel0.astype(np.int32)
    cum = np.cumsum(sel0_i, axis=0) - sel0_i                    # exclusive cumsum
    selected = sel0 & (cum < CAP)
    slot = cum
    used = selected.sum(axis=1)
    tok_of_slot = np.zeros(E * CAP, np.int32)
    valid = np.zeros(E * CAP, bool)
    b_idx, e_idx = np.nonzero(selected)
    flat = e_idx * CAP + slot[b_idx, e_idx]
    tok_of_slot[flat] = b_idx
    valid[flat] = True
    return tok_of_slot, valid, used, selected, slot


def _assign(routing):
    """Pair experts heavy-with-light across cores.

    Returns (order, WA, WB): order[c] = (expert_A, expert_B) for core c,
    widths WA >= WB are maxima over cores, rounded up to 8.
    """
    selected = routing[3]
    loads = selected.sum(axis=0)
    s = np.argsort(-loads, kind="stable")
    order = [(int(s[i]), int(s[E - 1 - i])) for i in range(N_CORES)]
    wa = max(int(loads[a]) for a, _ in order)
    wb = max(int(loads[b]) for _, b in order)
    wa = min(CAP, max(32, -(-wa // 8) * 8))
    wb = min(CAP, max(32, -(-wb // 8) * 8))
    return order, wa, wb


def _fp8(v):
    return v.astype(NPFP8)


def _hilo(vs):
    """Same-scale hi/lo fp8 split of (already scaled) values."""
    hi = _fp8(vs)
    lo = _fp8(vs - hi.astype(np.float32))
    return hi, lo


def _col_scale(W):
    return WTGT / np.maximum(np.abs(W).max(axis=0), 1e-30)


def _pack_w1(W1e, s1):
    # scaled W1 [D, F] -> [CH1, P, hl*kp*i*256]; row kp*256+i*128+p,
    # f block c*256+fb
    Ws = W1e * s1[None, :]
    hi, lo = _hilo(Ws)
    # [D, F] index: [kp(2), i(2), p(128), c(8), fb(256)]
    def arr(v):
        return v.reshape(KP, 2, P, CH1, M_PER * P).transpose(3, 2, 0, 1, 4)
    a = np.stack([arr(hi), arr(lo)], axis=2)  # [c, p, hl, kp, i, fb]
    return np.ascontiguousarray(a.reshape(CH1, P, -1))


def _pack_w2(W2e, s2):
    # scaled W2 [F, D] -> [CH2, P, hl*tpl*i*512]; row (2c+tpl)*256+i*128+p
    Ws = W2e * s2[None, :]
    hi, lo = _hilo(Ws)
    # [F, D] index: [c(4), tpl(2), i(2), p(128), d(512)]
    def arr(v):
        return v.reshape(CH2, T_PER, 2, P, D).transpose(0, 3, 1, 2, 4)
    a = np.stack([arr(hi), arr(lo)], axis=2)  # [c, p, hl, tpl, i, d]
    return np.ascontiguousarray(a.reshape(CH2, P, -1))


def _pack_xg(x, routing, expert, width):
    """Gather expert tokens -> [P, hl*kp*i*width] fp8 (scaled by SX)."""
    tok_of_slot, valid, used, selected, slot = routing
    sl = slice(expert * CAP, expert * CAP + width)
    xgE = x[tok_of_slot[sl]] * valid[sl, None].astype(np.float32)   # [w, D]
    xs = xgE.T * SX                                                 # [D, w]
    hi, lo = _hilo(xs)
    # [D, w] -> [kp, i, p, w] -> [p, hl, kp, i, w]
    def arr(v):
        return v.reshape(KP, 2, P, width)
    a = np.stack([arr(hi), arr(lo)], axis=1)    # [kp, hl, i?] careful below
    # arr gives [kp, i, p, w]; stack axis=1 -> [kp, hl, i, p, w]
    a = a.transpose(3, 1, 0, 2, 4)              # [p, hl, kp, i, w]
    return np.ascontiguousarray(a.reshape(P, -1))


def _prep_in_maps(x, W1, b1, W2, b2, routing, order, wa, wb, with_bias=True):
    widths = (wa, wb)
    NSC = 2 * (FT + KT)
    in_maps = []
    s1s = [_col_scale(W1[e]) for e in range(E)]
    s2s = [_col_scale(W2[e]) for e in range(E)]
    w1p = {e: _pack_w1(W1[e], s1s[e]) for e in range(E)}
    w2p = {e: _pack_w2(W2[e], s2s[e]) for e in range(E)}
    for c in range(N_CORES):
        es = order[c]
        m = {
            "w1": np.stack([w1p[e] for e in es]),
            "w2": np.stack([w2p[e] for e in es]),
        }
        for s, e in enumerate(es):
            m[f"xg{s}"] = _pack_xg(x, routing, e, widths[s])
        scs = np.zeros((P, EPC, NSC), np.float32)
        for s, e in enumerate(es):
            scs[:, s, :FT] = (1.0 / (SX * s1s[e])).reshape(FT, P).T
            scs[:, s, FT : FT + KT] = (1.0 / s2s[e]).reshape(KT, P).T
            if with_bias:
                scs[:, s, FT + KT : FT + KT + FT] = b1[e].reshape(FT, P).T
                scs[:, s, FT + KT + FT :] = b2[e].reshape(KT, P).T
        m["scs"] = np.ascontiguousarray(scs.reshape(P, -1))
        in_maps.append(m)
    return in_maps


def _erf(v):
    # Abramowitz & Stegun 7.1.26, |err| <= 1.5e-7
    s = np.sign(v)
    a = np.abs(v)
    t = 1.0 / (1.0 + 0.3275911 * a)
    poly = t * (
        0.254829592
        + t * (-0.284496736 + t * (1.421413741 + t * (-1.453152027 + t * 1.061405429)))
    )
    return s * (1.0 - poly * np.exp(-a * a))


def _gelu_exact(v):
    return 0.5 * v * (1.0 + _erf(v / np.sqrt(2.0)))


def kernel(x, W1, b1, W2, b2, Wf1, bf1, Wf2, bf2, routes):
    x = np.asarray(x, np.float32)
    W1 = np.asarray(W1, np.float32)
    b1 = np.asarray(b1, np.float32)
    W2 = np.asarray(W2, np.float32)
    b2 = np.asarray(b2, np.float32)
    Wf1 = np.asarray(Wf1, np.float32)
    bf1 = np.asarray(bf1, np.float32)
    Wf2 = np.asarray(Wf2, np.float32)
    bf2 = np.asarray(bf2, np.float32)
    routes = np.asarray(routes)

    routing = _route_np(routes)
    tok_of_slot, valid, used, selected, slot = routing
    order, wa, wb = _assign(routing)
    with_bias = bool(np.any(b1) or np.any(b2))
    in_maps = _prep_in_maps(x, W1, b1, W2, b2, routing, order, wa, wb, with_bias)

    nc = get_nc(wa=wa, wb=wb, with_bias=with_bias)
    res = run_bass_kernel_spmd(nc, in_maps, core_ids=list(range(N_CORES)))

    # Per-expert outputs [D, width] f32 (garbage in invalid slots; never
    # read there).
    widths = (wa, wb)
    exp_out = [None] * E
    for c in range(N_CORES):
        for s, e in enumerate(order[c]):
            Yc = res.results[c][f"yt{s}"].astype(np.float32)
            exp_out[e] = Yc.reshape(P, KT, widths[s]).transpose(1, 0, 2).reshape(
                D, widths[s]
            )

    # Combine: each token was selected by <= 2 experts; gather its slot
    # outputs and average. Pure host-side gather.
    out = np.zeros((B, D), np.float32)
    b_idx, e_idx = np.nonzero(selected)                         # ordered by token
    s_of = slot[b_idx, e_idx]
    for e in range(E):
        msk = e_idx == e
        out[b_idx[msk]] += exp_out[e][:, s_of[msk]].T
    inv = (1.0 / np.maximum(used, 1)).astype(np.float32)
    out *= inv[:, None]

    # Overflow tokens (used == 0): exact fallback FFN on host.
    ovf = np.nonzero(used == 0)[0]
    if ovf.size:
        xo = x[ovf]
        fb = _gelu_exact(xo @ Wf1 + bf1) @ Wf2 + bf2
        out[ovf] = fb.astype(np.float32)

    return out.astype(np.float32)


# revision 19
# speedup vs baseline: 1.1480x; 1.1299x over previous
"""MoE FFN (capacity-gated routing) on 8 Trainium2 NeuronCores.

Strategy
--------
Expert-parallel: 16 experts / 8 cores = 2 experts per core; host-side
routing/dispatch/combine (full-I/O contract makes the dispatch gather
the sharding step). Experts are sorted by load and paired
heavy-with-light across cores, so the SPMD program uses two static
token widths (WA, WB) = per-slot maxima over cores.

v3: the grouped FFN runs on fp8 (float8e4 = e4m3) with DoubleRow
matmuls (2 contraction rows per partition, 0.5 cycles/row -> 4x bf16
column throughput) while keeping ~bf16 accuracy via an error-split:
every operand is represented as hi + lo fp8 terms sharing one scale
(x, W1, W2 split on the host; h split on-device: ACT writes f32 h,
DVE casts hh=fp8(h) and hl=fp8(h-hh)). Each matmul accumulates the
three significant cross terms (hi*hi + hi*lo + lo*hi) into one PSUM
group; the dropped lo*lo term is O(3e-4) relative. Per-column weight
scales (folded with the global x scale) are applied as per-partition
ScalarE activation scales on PSUM readout, so dequantization is free.
Weight bytes stay 2/element (same DMA as bf16) but PE time drops 25%,
moving the kernel from PE-bound to the HBM roofline. Expert outputs
return as bf16.
"""

import sys

if "/opt/trn_rl_repo" not in sys.path:
    sys.path.append("/opt/trn_rl_repo")

import numpy as np
import ml_dtypes

import concourse.tile as tile
from concourse import bacc, mybir
from concourse.bass_utils import run_bass_kernel_spmd

# Problem shape (hardcoded per contract)
D = 512        # d_model
F = 2048       # d_ff
E = 16         # num experts
B = 2048       # max tokens
CAP = 320      # per-expert capacity = int(1.25 * ceil(B * 2 / E))
N_CORES = 8
EPC = E // N_CORES  # experts per core

P = 128
KT = D // P    # k-tiles over d_model (4)
KP = KT // 2   # DoubleRow k-pair instructions per term (2)
FT = F // P    # tiles over d_ff (16)
TP = FT // 2   # DoubleRow t-pair instructions per term in phase 2 (8)
CH1 = 8        # w1 chunks along f (2 f-tiles each)
CH2 = 8        # w2 chunks along t-pairs (1 t-pair each; the last chunk
               # then gates only 3 matmuls per output group in the tail)
M_PER = FT // CH1   # f-tiles per w1 chunk (2)
T_PER = TP // CH2   # t-pairs per w2 chunk (2)

SX = 16.0      # global x scale into fp8
WTGT = 224.0   # per-column weight scale target (e4m3 max 448/240; stay safe)

FP8 = mybir.dt.float8e4
BF16 = mybir.dt.bfloat16
F32 = mybir.dt.float32
NPFP8 = ml_dtypes.float8_e4m3
NPBF16 = ml_dtypes.bfloat16

DR = mybir.MatmulPerfMode.DoubleRow

_CACHE = {}


def _build_nc(rep=1, act=None, wa=CAP, wb=CAP, with_bias=True):
    """Per-core program: grouped FFN for 2 experts (widths wa >= wb).

    fp8 hi/lo DoubleRow pipeline; see module docstring. All tensors are
    d-major; weights/xg arrive pre-packed in per-partition byte order
    (see _prep_in_maps) so every DMA is [128 x contiguous].
    """
    if act is None:
        act = mybir.ActivationFunctionType.Gelu
    nc = bacc.Bacc(None)
    widths = (wa, wb)
    # xg{s}: [P, hl(2), kp(2), i(2), w] fp8; row (kp*256 + i*128 + p) of
    # scaled x, hi term then lo term.
    xg = [
        nc.declare_dram_parameter(f"xg{s}", [P, 8 * widths[s]], FP8, isOutput=False)
        for s in range(EPC)
    ]
    # w1[e, c]: [P, hl(2), kp(2), i(2), 256] fp8 covering f cols
    # [c*256, (c+1)*256) of scaled W1.
    w1 = nc.declare_dram_parameter(
        "w1", [EPC, CH1, P, 8 * M_PER * P], FP8, isOutput=False
    )
    # w2[e, c]: [P, hl(2), tpl(2), i(2), 512] fp8 covering contraction
    # rows [(2c)*256, (2c+2)*256) of scaled W2.
    w2 = nc.declare_dram_parameter(
        "w2", [EPC, CH2, P, 4 * T_PER * D], FP8, isOutput=False
    )
    # scs: per-partition dequant scales (+ biases when present):
    # [P, s*(2*(FT+KT)) + m]: m<FT: 1/(SX*s1); m<FT+KT: 1/s2;
    # then FT+KT bias entries (b1, b2) in the same order.
    NSC = 2 * (FT + KT)
    scs = nc.declare_dram_parameter("scs", [P, EPC * NSC], F32, isOutput=False)
    yt = [
        nc.declare_dram_parameter(f"yt{s}", [P, KT * widths[s]], BF16, isOutput=True)
    for s in range(EPC)
    ]

    with (
        tile.TileContext(nc) as tc,
        tc.tile_pool(name="consts", bufs=1) as consts,
        tc.tile_pool(name="xgp", bufs=2) as xgp,
        tc.tile_pool(name="w1p", bufs=2 * CH1 + 1) as w1p,
        tc.tile_pool(name="w2p", bufs=2 * CH2 + 1) as w2p,
        tc.tile_pool(name="hp", bufs=2) as hp,
        tc.tile_pool(name="hhp", bufs=2) as hhp,
        tc.tile_pool(name="hlp", bufs=2) as hlp,
        tc.tile_pool(name="yp", bufs=2) as yp,
        tc.tile_pool(name="ps1", bufs=4, space="PSUM") as ps1,
        tc.tile_pool(name="ps2", bufs=4, space="PSUM") as ps2,
    ):
        scs_sb = consts.tile([P, EPC, NSC], F32, name="scs_sb")
        scs_issued = False

        # PE warm-up: dummy matmuls on a zeroed tile keep the PE busy (and
        # the p-state ramp running) while the first xg/w1 chunks stream in.
        zt = consts.tile([P, max(P, wa)], BF16, name="warm_zt")
        nc.vector.memset(zt, 0.0)
        for _w in range(6):
            pw = ps1.tile([P, wa], F32, tag="ps", name="warm_ps")
            nc.tensor.matmul(pw, zt[:, :P], zt[:, :wa], start=True, stop=True)

        # DMA queue split: same-queue transfers serialize (in-order SEQ)
        # but different engine queues' transfers overlap fully, so the
        # 9MB stream is spread over SP / ACT / Pool. ACT and DVE carry
        # heavy compute (gelu + h-split), so they only get transfers
        # scheduled where their pipelines are otherwise idle.
        def issue_w1(e, c, eng=None):
            w1c = w1p.tile([P, 2, KP, 2, M_PER * P], FP8, tag="w1c", name="w1c")
            (eng or nc.sync).dma_start(
                out=w1c,
                in_=w1[e, c].rearrange(
                    "p (hl kp i f) -> p hl kp i f", hl=2, kp=KP, i=2
                ),
            )
            return w1c

        def issue_xg(e, eng=None):
            nb = widths[e]
            xg_sb = xgp.tile([P, 2, KP, 2, nb], FP8, name="xg_sb")
            (eng or nc.sync).dma_start(
                out=xg_sb,
                in_=xg[e].rearrange("p (hl kp i n) -> p hl kp i n", hl=2, kp=KP, i=2),
            )
            return xg_sb

        prefetched = {}
        deferred_yt = [None]

        for _ in range(rep):
            for e in range(EPC):
                nb = widths[e]
                if e in prefetched:
                    xg_sb, w1c0 = prefetched.pop(e)
                else:
                    # the two first-matmul inputs go on different queues so
                    # their transfers overlap (ACT is act-free this early)
                    w1c0 = issue_w1(e, 0, eng=nc.scalar)
                    xg_sb = issue_xg(e)
                w1cs = [w1c0]
                h_sb = hp.tile([P, FT, nb], F32, name="h_sb")
                hh_sb = hhp.tile([P, FT, nb], FP8, name="hh_sb")
                hl_sb = hlp.tile([P, FT, nb], FP8, name="hl_sb")

                # phase 1: h = gelu((x @ W1) * scale1); 3 DR terms per psum
                # group; w1 chunked along f so matmuls start after one
                # 256KB chunk lands. The scs DMA and the previous expert's
                # deferred yt store slot into the chunk stream so small
                # transfers never cluster at the head of the HWDGE queue.
                for c in range(CH1):
                    if c > 0:
                        # expert 0's odd chunks ride the Pool queue (idle
                        # until w2 loads begin) so phase 1 is never
                        # chunk-starved; expert 1's stream has slack on SP
                        eng = nc.gpsimd if (e == 0 and c % 2 == 1) else None
                        w1cs.append(issue_w1(e, c, eng=eng))
                    if c == 3 and deferred_yt[0] is not None:
                        deferred_yt[0]()
                        deferred_yt[0] = None
                    w1c = w1cs[c]
                    for ml in range(M_PER):
                        m = c * M_PER + ml
                        ps = ps1.tile([P, nb], F32, name="ps")
                        first = True
                        for whl, xhl in ((0, 0), (1, 0), (0, 1)):
                            for kp in range(KP):
                                nc.tensor.matmul(
                                    ps,
                                    w1c[:, whl, kp, :, ml * P : (ml + 1) * P],
                                    xg_sb[:, xhl, kp, :, :],
                                    start=first,
                                    stop=(whl == 0 and xhl == 1 and kp == KP - 1),
                                    perf_mode=DR,
                                )
                                first = False
                        if not scs_issued:
                            # traced before the first ACT (deps follow trace
                            # order) but issued via the Pool-engine SWDGE
                            # queue so it doesn't take an HWDGE slot from
                            # the weight-chunk stream
                            nc.gpsimd.dma_start(
                                out=scs_sb,
                                in_=scs.rearrange("p (e t) -> p e t", e=EPC),
                            )
                            scs_issued = True
                        nc.scalar.activation(
                            h_sb[:, m, :],
                            ps,
                            act,
                            bias=(
                                scs_sb[:, e, FT + KT + m : FT + KT + m + 1]
                                if with_bias
                                else 0.0
                            ),
                            scale=scs_sb[:, e, m : m + 1],
                        )
                        nc.vector.tensor_copy(out=hh_sb[:, m, :], in_=h_sb[:, m, :])
                        nc.vector.tensor_sub(
                            hl_sb[:, m, :], h_sb[:, m, :], hh_sb[:, m, :]
                        )

                # phase 2: y = (h @ W2) * scale2; tp-outer so each w2 chunk
                # is consumed on arrival; 4 output groups accumulate in
                # parallel across the 4 ps2 banks
                ps_y = [
                    ps2.tile([P, nb], F32, tag="psy", name=f"psy{m2}")
                    for m2 in range(KT)
                ]
                w2cs = []
                for c in range(CH2):
                    w2c = w2p.tile([P, 2, T_PER, 2, D], FP8, tag="w2c", name="w2c")
                    nc.gpsimd.dma_start(
                        out=w2c,
                        in_=w2[e, c].rearrange(
                            "p (hl tpl i d) -> p hl tpl i d", hl=2, tpl=T_PER, i=2
                        ),
                    )
                    w2cs.append(w2c)
                    if c == 3 and e + 1 < EPC:
                        # prefetch the next expert's first-matmul inputs so
                        # its phase 1 starts as soon as this phase 2 ends
                        nw1c0 = issue_w1(e + 1, 0)
                        nxg = issue_xg(e + 1, eng=nc.gpsimd)
                        prefetched[e + 1] = (nxg, nw1c0)

                def mm2(tp, m2):
                    c, tpl = divmod(tp, T_PER)
                    first = True
                    for whl, hsrc in ((0, hh_sb), (1, hh_sb), (0, hl_sb)):
                        nc.tensor.matmul(
                            ps_y[m2],
                            w2cs[c][:, whl, tpl, :, m2 * P : (m2 + 1) * P],
                            hsrc[:, 2 * tp : 2 * tp + 2, :],
                            start=(tp == 0 and first),
                            stop=(tp == TP - 1 and whl == 0 and hsrc is hl_sb),
                            perf_mode=DR,
                        )
                        first = False

                y_sb = yp.tile([P, KT, nb], BF16, name="y_sb")

                def act_y(m2, on_dve=False):
                    if on_dve and not with_bias:
                        # spread the final dequant readouts across ACT and
                        # DVE so the last groups retire in parallel chains
                        nc.vector.tensor_scalar_mul(
                            y_sb[:, m2, :],
                            ps_y[m2],
                            scs_sb[:, e, FT + m2 : FT + m2 + 1],
                        )
                        return
                    nc.scalar.activation(
                        y_sb[:, m2, :],
                        ps_y[m2],
                        mybir.ActivationFunctionType.Identity,
                        bias=(
                            scs_sb[
                                :, e, FT + KT + FT + m2 : FT + KT + FT + m2 + 1
                            ]
                            if with_bias
                            else 0.0
                        ),
                        scale=scs_sb[:, e, FT + m2 : FT + m2 + 1],
                    )

                last = e == EPC - 1
                yt_v = yt[e].rearrange("p (t n) -> p t n", t=KT)
                for tp in range(TP - 1 if last else TP):
                    for m2 in range(KT):
                        mm2(tp, m2)
                if not last:
                    for m2 in range(KT):
                        act_y(m2)

                    def defer(yv=yt_v, ys=y_sb):
                        nc.sync.dma_start(out=yv, in_=ys)

                    deferred_yt[0] = defer
                else:
                    # retire each group on only its 3 last-t-pair matmuls:
                    # the final w2 chunk (the last byte off HBM) gates just
                    # 12 matmuls total, and the acts/yt DMAs stagger behind
                    # it group by group (pieces >=992B/partition, full-rate)
                    for m2 in range(KT):
                        mm2(TP - 1, m2)
                        act_y(m2, on_dve=(m2 % 2 == 1))
                        if m2 == 1:
                            nc.sync.dma_start(
                                out=yt_v[:, 0:2, :], in_=y_sb[:, 0:2, :]
                            )
                        elif m2 == 3:
                            # different queue than the first piece so the
                            # two terminal stores overlap
                            nc.scalar.dma_start(
                                out=yt_v[:, 2:4, :], in_=y_sb[:, 2:4, :]
                            )
    nc.finalize()
    return nc


def get_nc(rep=1, act=None, wa=CAP, wb=CAP, with_bias=True):
    key = (rep, act, wa, wb, with_bias)
    if key not in _CACHE:
        _CACHE[key] = _build_nc(rep, act, wa, wb, with_bias)
    return _CACHE[key]


def _route_np(routes):
    """Numpy replica of the reference's capacity-gated routing."""
    e_map = (routes.astype(np.int64) * E) // B                  # [B, K]
    sel0 = np.zeros((B, E), bool)
    np.put_along_axis(sel0, e_map, True, axis=1)
    sel0_i = sel0.astype(np.int32)
    cum = np.cumsum(sel0_i, axis=0) - sel0_i                    # exclusive cumsum
    selected = sel0 & (cum < CAP)
    slot = cum
    used = selected.sum(axis=1)
    tok_of_slot = np.zeros(E * CAP, np.int32)
    valid = np.zeros(E * CAP, bool)
    b_idx, e_idx = np.nonzero(selected)
    flat = e_idx * CAP + slot[b_idx, e_idx]
    tok_of_slot[flat] = b_idx
    valid[flat] = True
    return tok_of_slot, valid, used, selected, slot


def _assign(routing):
    """Pair experts heavy-with-light across cores.

    Returns (order, WA, WB): order[c] = (expert_A, expert_B) for core c,
    widths WA >= WB are maxima over cores, rounded up to 8.
    """
    selected = routing[3]
    loads = selected.sum(axis=0)
    s = np.argsort(-loads, kind="stable")
    order = [(int(s[i]), int(s[E - 1 - i])) for i in range(N_CORES)]
    wa = max(int(loads[a]) for a, _ in order)
    wb = max(int(loads[b]) for _, b in order)
    wa = min(CAP, max(32, -(-wa // 8) * 8))
    wb = min(CAP, max(32, -(-wb // 8) * 8))
    return order, wa, wb


def _fp8(v):
    return v.astype(NPFP8)


def _hilo(vs):
    """Same-scale hi/lo fp8 split of (already scaled) values."""
    hi = _fp8(vs)
    lo = _fp8(vs - hi.astype(np.float32))
    return hi, lo


def _col_scale(W):
    return WTGT / np.maximum(np.abs(W).max(axis=0), 1e-30)


def _pack_w1(W1e, s1):
    # scaled W1 [D, F] -> [CH1, P, hl*kp*i*256]; row kp*256+i*128+p,
    # f block c*256+fb
    Ws = W1e * s1[None, :]
    hi, lo = _hilo(Ws)
    # [D, F] index: [kp(2), i(2), p(128), c(8), fb(256)]
    def arr(v):
        return v.reshape(KP, 2, P, CH1, M_PER * P).transpose(3, 2, 0, 1, 4)
    a = np.stack([arr(hi), arr(lo)], axis=2)  # [c, p, hl, kp, i, fb]
    return np.ascontiguousarray(a.reshape(CH1, P, -1))


def _pack_w2(W2e, s2):
    # scaled W2 [F, D] -> [CH2, P, hl*tpl*i*512]; row (2c+tpl)*256+i*128+p
    Ws = W2e * s2[None, :]
    hi, lo = _hilo(Ws)
    # [F, D] index: [c(4), tpl(2), i(2), p(128), d(512)]
    def arr(v):
        return v.reshape(CH2, T_PER, 2, P, D).transpose(0, 3, 1, 2, 4)
    a = np.stack([arr(hi), arr(lo)], axis=2)  # [c, p, hl, tpl, i, d]
    return np.ascontiguousarray(a.reshape(CH2, P, -1))


def _pack_xg(x, routing, expert, width):
    """Gather expert tokens -> [P, hl*kp*i*width] fp8 (scaled by SX)."""
    tok_of_slot, valid, used, selected, slot = routing
    sl = slice(expert * CAP, expert * CAP + width)
    xgE = x[tok_of_slot[sl]] * valid[sl, None].astype(np.float32)   # [w, D]
    xs = xgE.T * SX                                                 # [D, w]
    hi, lo = _hilo(xs)
    # [D, w] -> [kp, i, p, w] -> [p, hl, kp, i, w]
    def arr(v):
        return v.reshape(KP, 2, P, width)
    a = np.stack([arr(hi), arr(lo)], axis=1)    # [kp, hl, i?] careful below
    # arr gives [kp, i, p, w]; stack axis=1 -> [kp, hl, i, p, w]
    a = a.transpose(3, 1, 0, 2, 4)              # [p, hl, kp, i, w]
    return np.ascontiguousarray(a.reshape(P, -1))


def _prep_in_maps(x, W1, b1, W2, b2, routing, order, wa, wb, with_bias=True):
    widths = (wa, wb)
    NSC = 2 * (FT + KT)
    in_maps = []
    s1s = [_col_scale(W1[e]) for e in range(E)]
    s2s = [_col_scale(W2[e]) for e in range(E)]
    w1p = {e: _pack_w1(W1[e], s1s[e]) for e in range(E)}
    w2p = {e: _pack_w2(W2[e], s2s[e]) for e in range(E)}
    for c in range(N_CORES):
        es = order[c]
        m = {
            "w1": np.stack([w1p[e] for e in es]),
            "w2": np.stack([w2p[e] for e in es]),
        }
        for s, e in enumerate(es):
            m[f"xg{s}"] = _pack_xg(x, routing, e, widths[s])
        scs = np.zeros((P, EPC, NSC), np.float32)
        for s, e in enumerate(es):
            scs[:, s, :FT] = (1.0 / (SX * s1s[e])).reshape(FT, P).T
            scs[:, s, FT : FT + KT] = (1.0 / s2s[e]).reshape(KT, P).T
            if with_bias:
                scs[:, s, FT + KT : FT + KT + FT] = b1[e].reshape(FT, P).T
                scs[:, s, FT + KT + FT :] = b2[e].reshape(KT, P).T
        m["scs"] = np.ascontiguousarray(scs.reshape(P, -1))
        in_maps.append(m)
    return in_maps


def _erf(v):
    # Abramowitz & Stegun 7.1.26, |err| <= 1.5e-7
    s = np.sign(v)
    a = np.abs(v)
    t = 1.0 / (1.0 + 0.3275911 * a)
    poly = t * (
        0.254829592
        + t * (-0.284496736 + t * (1.421413741 + t * (-1.453152027 + t * 1.061405429)))
    )
    return s * (1.0 - poly * np.exp(-a * a))


def _gelu_exact(v):
    return 0.5 * v * (1.0 + _erf(v / np.sqrt(2.0)))


def kernel(x, W1, b1, W2, b2, Wf1, bf1, Wf2, bf2, routes):
    x = np.asarray(x, np.float32)
    W1 = np.asarray(W1, np.float32)
    b1 = np.asarray(b1, np.float32)
    W2 = np.asarray(W2, np.float32)
    b2 = np.asarray(b2, np.float32)
    Wf1 = np.asarray(Wf1, np.float32)
    bf1 = np.asarray(bf1, np.float32)
    Wf2 = np.asarray(Wf2, np.float32)
    bf2 = np.asarray(bf2, np.float32)
    routes = np.asarray(routes)

    routing = _route_np(routes)
    tok_of_slot, valid, used, selected, slot = routing
    order, wa, wb = _assign(routing)
    with_bias = bool(np.any(b1) or np.any(b2))
    in_maps = _prep_in_maps(x, W1, b1, W2, b2, routing, order, wa, wb, with_bias)

    nc = get_nc(wa=wa, wb=wb, with_bias=with_bias)
    res = run_bass_kernel_spmd(nc, in_maps, core_ids=list(range(N_CORES)))

    # Per-expert outputs [D, width] f32 (garbage in invalid slots; never
    # read there).
    widths = (wa, wb)
    exp_out = [None] * E
    for c in range(N_CORES):
        for s, e in enumerate(order[c]):
            Yc = res.results[c][f"yt{s}"].astype(np.float32)
            exp_out[e] = Yc.reshape(P, KT, widths[s]).transpose(1, 0, 2).reshape(
                D, widths[s]
            )

    # Combine: each token was selected by <= 2 experts; gather its slot
    # outputs and average. Pure host-side gather.
    out = np.zeros((B, D), np.float32)
    b_idx, e_idx = np.nonzero(selected)                         # ordered by token
    s_of = slot[b_idx, e_idx]
    for e in range(E):
        msk = e_idx == e
        out[b_idx[msk]] += exp_out[e][:, s_of[msk]].T
    inv = (1.0 / np.maximum(used, 1)).astype(np.float32)
    out *= inv[:, None]

    # Overflow tokens (used == 0): exact fallback FFN on host.
    ovf = np.nonzero(used == 0)[0]
    if ovf.size:
        xo = x[ovf]
        fb = _gelu_exact(xo @ Wf1 + bf1) @ Wf2 + bf2
        out[ovf] = fb.astype(np.float32)

    return out.astype(np.float32)


# revision 22
# speedup vs baseline: 1.1792x; 1.0272x over previous
"""MoE FFN (capacity-gated routing) on 8 Trainium2 NeuronCores.

Strategy
--------
Expert-parallel: 16 experts / 8 cores = 2 experts per core; host-side
routing/dispatch/combine (full-I/O contract makes the dispatch gather
the sharding step). Experts are sorted by load and paired
heavy-with-light across cores, so the SPMD program uses two static
token widths (WA, WB) = per-slot maxima over cores.

v3: the grouped FFN runs on fp8 (float8e4 = e4m3) with DoubleRow
matmuls (2 contraction rows per partition, 0.5 cycles/row -> 4x bf16
column throughput) while keeping ~bf16 accuracy via an error-split:
every operand is represented as hi + lo fp8 terms sharing one scale
(x, W1, W2 split on the host; h split on-device: ACT writes f32 h,
DVE casts hh=fp8(h) and hl=fp8(h-hh)). Each matmul accumulates the
three significant cross terms (hi*hi + hi*lo + lo*hi) into one PSUM
group; the dropped lo*lo term is O(3e-4) relative. Per-column weight
scales (folded with the global x scale) are applied as per-partition
ScalarE activation scales on PSUM readout, so dequantization is free.
Weight bytes stay 2/element (same DMA as bf16) but PE time drops 25%,
moving the kernel from PE-bound to the HBM roofline. Expert outputs
return as bf16.
"""

import sys

if "/opt/trn_rl_repo" not in sys.path:
    sys.path.append("/opt/trn_rl_repo")

import numpy as np
import ml_dtypes

import concourse.tile as tile
from concourse import bacc, mybir
from concourse.bass_utils import run_bass_kernel_spmd

# Problem shape (hardcoded per contract)
D = 512        # d_model
F = 2048       # d_ff
E = 16         # num experts
B = 2048       # max tokens
CAP = 320      # per-expert capacity = int(1.25 * ceil(B * 2 / E))
N_CORES = 8
EPC = E // N_CORES  # experts per core

P = 128
KT = D // P    # k-tiles over d_model (4)
KP = KT // 2   # DoubleRow k-pair instructions per term (2)
FT = F // P    # tiles over d_ff (16)
TP = FT // 2   # DoubleRow t-pair instructions per term in phase 2 (8)
CH1 = 8        # w1 chunks along f (2 f-tiles each)
CH2 = 8        # w2 chunks along t-pairs (1 t-pair each; the last chunk
               # then gates only 3 matmuls per output group in the tail)
M_PER = FT // CH1   # f-tiles per w1 chunk (2)
T_PER = TP // CH2   # t-pairs per w2 chunk (2)

SX = 16.0      # global x scale into fp8
WTGT = 224.0   # per-column weight scale target (e4m3 max 448/240; stay safe)

FP8 = mybir.dt.float8e4
BF16 = mybir.dt.bfloat16
F32 = mybir.dt.float32
NPFP8 = ml_dtypes.float8_e4m3
NPBF16 = ml_dtypes.bfloat16

DR = mybir.MatmulPerfMode.DoubleRow

_CACHE = {}


def _build_nc(rep=1, act=None, wa=CAP, wb=CAP, with_bias=True):
    """Per-core program: grouped FFN for 2 experts (widths wa >= wb).

    fp8 hi/lo DoubleRow pipeline; see module docstring. All tensors are
    d-major; weights/xg arrive pre-packed in per-partition byte order
    (see _prep_in_maps) so every DMA is [128 x contiguous].
    """
    if act is None:
        act = mybir.ActivationFunctionType.Gelu
    nc = bacc.Bacc(None)
    widths = (wa, wb)
    # xg{s}: [P, hl(2), kp(2), i(2), w] fp8; row (kp*256 + i*128 + p) of
    # scaled x, hi term then lo term.
    xg = [
        nc.declare_dram_parameter(f"xg{s}", [P, 8 * widths[s]], FP8, isOutput=False)
        for s in range(EPC)
    ]
    # w1[e, c]: [P, hl(2), kp(2), i(2), 256] fp8 covering f cols
    # [c*256, (c+1)*256) of scaled W1.
    w1 = nc.declare_dram_parameter(
        "w1", [EPC, CH1, P, 8 * M_PER * P], FP8, isOutput=False
    )
    # w2[e, c]: [P, hl(2), tpl(2), i(2), 512] fp8 covering contraction
    # rows [(2c)*256, (2c+2)*256) of scaled W2.
    w2 = nc.declare_dram_parameter(
        "w2", [EPC, CH2, P, 4 * T_PER * D], FP8, isOutput=False
    )
    # scs: per-partition dequant scales (+ biases when present):
    # [P, s*(2*(FT+KT)) + m]: m<FT: 1/(SX*s1); m<FT+KT: 1/s2;
    # then FT+KT bias entries (b1, b2) in the same order.
    NSC = 2 * (FT + KT)
    scs = nc.declare_dram_parameter("scs", [P, EPC * NSC], F32, isOutput=False)
    yt = [
        nc.declare_dram_parameter(f"yt{s}", [P, KT * widths[s]], BF16, isOutput=True)
    for s in range(EPC)
    ]

    with (
        tile.TileContext(nc) as tc,
        tc.tile_pool(name="consts", bufs=1) as consts,
        tc.tile_pool(name="xgp", bufs=2) as xgp,
        tc.tile_pool(name="w1p", bufs=2 * CH1 + 1) as w1p,
        tc.tile_pool(name="w2p", bufs=2 * CH2 + 1) as w2p,
        tc.tile_pool(name="hp", bufs=2) as hp,
        tc.tile_pool(name="hhp", bufs=2) as hhp,
        tc.tile_pool(name="hlp", bufs=2) as hlp,
        tc.tile_pool(name="yp", bufs=2) as yp,
        tc.tile_pool(name="ps1", bufs=4, space="PSUM") as ps1,
        tc.tile_pool(name="ps2", bufs=4, space="PSUM") as ps2,
    ):
        scs_sb = consts.tile([P, EPC, NSC], F32, name="scs_sb")
        scs_issued = False

        # PE warm-up: dummy matmuls on a zeroed tile keep the PE busy (and
        # the p-state ramp running) while the first xg/w1 chunks stream in.
        zt = consts.tile([P, max(P, wa)], BF16, name="warm_zt")
        nc.vector.memset(zt, 0.0)
        for _w in range(11):
            pw = ps1.tile([P, wa], F32, tag="ps", name="warm_ps")
            nc.tensor.matmul(pw, zt[:, :P], zt[:, :wa], start=True, stop=True)

        # DMA queue split: same-queue transfers serialize (in-order SEQ)
        # but different engine queues' transfers overlap fully, so the
        # 9MB stream is spread over SP / ACT / Pool. ACT and DVE carry
        # heavy compute (gelu + h-split), so they only get transfers
        # scheduled where their pipelines are otherwise idle.
        def issue_w1(e, c, eng=None):
            w1c = w1p.tile([P, 2, KP, 2, M_PER * P], FP8, tag="w1c", name="w1c")
            (eng or nc.sync).dma_start(
                out=w1c,
                in_=w1[e, c].rearrange(
                    "p (hl kp i f) -> p hl kp i f", hl=2, kp=KP, i=2
                ),
            )
            return w1c

        def issue_xg(e, eng=None):
            nb = widths[e]
            xg_sb = xgp.tile([P, 2, KP, 2, nb], FP8, name="xg_sb")
            (eng or nc.sync).dma_start(
                out=xg_sb,
                in_=xg[e].rearrange("p (hl kp i n) -> p hl kp i n", hl=2, kp=KP, i=2),
            )
            return xg_sb

        prefetched = {}
        deferred_yt = [None]

        for _ in range(rep):
            for e in range(EPC):
                nb = widths[e]
                if e in prefetched:
                    xg_sb, w1c0 = prefetched.pop(e)
                else:
                    # both first-matmul inputs ride SP back-to-back: ACT's
                    # queue head holds the 1.3us act-table load and Pool's
                    # SWDGE generation is ~1us, so SP-serial is still the
                    # fastest path to the first matmul
                    w1c0 = issue_w1(e, 0)
                    xg_sb = issue_xg(e)
                w1cs = [w1c0]
                h_sb = hp.tile([P, FT, nb], F32, name="h_sb")
                hh_sb = hhp.tile([P, FT, nb], FP8, name="hh_sb")
                hl_sb = hlp.tile([P, FT, nb], FP8, name="hl_sb")

                # phase 1: h = gelu((x @ W1) * scale1); 3 DR terms per psum
                # group; w1 chunked along f so matmuls start after one
                # 256KB chunk lands. The scs DMA and the previous expert's
                # deferred yt store slot into the chunk stream so small
                # transfers never cluster at the head of the HWDGE queue.
                for c in range(CH1):
                    if c > 0:
                        # expert 0's odd chunks ride the Pool queue (idle
                        # until w2 loads begin) so phase 1 is never
                        # chunk-starved; expert 1's stream has slack on SP
                        eng = nc.gpsimd if (e == 0 and c % 2 == 1) else None
                        w1cs.append(issue_w1(e, c, eng=eng))
                    if c == 3 and deferred_yt[0] is not None:
                        deferred_yt[0]()
                        deferred_yt[0] = None
                    w1c = w1cs[c]
                    for ml in range(M_PER):
                        m = c * M_PER + ml
                        ps = ps1.tile([P, nb], F32, name="ps")
                        first = True
                        for whl, xhl in ((0, 0), (1, 0), (0, 1)):
                            for kp in range(KP):
                                nc.tensor.matmul(
                                    ps,
                                    w1c[:, whl, kp, :, ml * P : (ml + 1) * P],
                                    xg_sb[:, xhl, kp, :, :],
                                    start=first,
                                    stop=(whl == 0 and xhl == 1 and kp == KP - 1),
                                    perf_mode=DR,
                                )
                                first = False
                        if not scs_issued:
                            # traced before the first ACT (deps follow trace
                            # order) but issued via the Pool-engine SWDGE
                            # queue so it doesn't take an HWDGE slot from
                            # the weight-chunk stream
                            nc.gpsimd.dma_start(
                                out=scs_sb,
                                in_=scs.rearrange("p (e t) -> p e t", e=EPC),
                            )
                            scs_issued = True
                        nc.scalar.activation(
                            h_sb[:, m, :],
                            ps,
                            act,
                            bias=(
                                scs_sb[:, e, FT + KT + m : FT + KT + m + 1]
                                if with_bias
                                else 0.0
                            ),
                            scale=scs_sb[:, e, m : m + 1],
                        )
                        nc.vector.tensor_copy(out=hh_sb[:, m, :], in_=h_sb[:, m, :])
                        nc.vector.tensor_sub(
                            hl_sb[:, m, :], h_sb[:, m, :], hh_sb[:, m, :]
                        )

                # phase 2: y = (h @ W2) * scale2; tp-outer so each w2 chunk
                # is consumed on arrival; 4 output groups accumulate in
                # parallel across the 4 ps2 banks
                ps_y = [
                    ps2.tile([P, nb], F32, tag="psy", name=f"psy{m2}")
                    for m2 in range(KT)
                ]
                w2cs = []
                for c in range(CH2):
                    w2c = w2p.tile([P, 2, T_PER, 2, D], FP8, tag="w2c", name="w2c")
                    nc.gpsimd.dma_start(
                        out=w2c,
                        in_=w2[e, c].rearrange(
                            "p (hl tpl i d) -> p hl tpl i d", hl=2, tpl=T_PER, i=2
                        ),
                    )
                    w2cs.append(w2c)
                    if c == 3 and e + 1 < EPC:
                        # prefetch the next expert's first-matmul inputs so
                        # its phase 1 starts as soon as this phase 2 ends
                        nw1c0 = issue_w1(e + 1, 0)
                        nxg = issue_xg(e + 1, eng=nc.gpsimd)
                        prefetched[e + 1] = (nxg, nw1c0)

                def mm2(tp, m2):
                    c, tpl = divmod(tp, T_PER)
                    first = True
                    for whl, hsrc in ((0, hh_sb), (1, hh_sb), (0, hl_sb)):
                        nc.tensor.matmul(
                            ps_y[m2],
                            w2cs[c][:, whl, tpl, :, m2 * P : (m2 + 1) * P],
                            hsrc[:, 2 * tp : 2 * tp + 2, :],
                            start=(tp == 0 and first),
                            stop=(tp == TP - 1 and whl == 0 and hsrc is hl_sb),
                            perf_mode=DR,
                        )
                        first = False

                y_sb = yp.tile([P, KT, nb], BF16, name="y_sb")

                def act_y(m2, on_dve=False):
                    if on_dve and not with_bias:
                        # spread the final dequant readouts across ACT and
                        # DVE so the last groups retire in parallel chains
                        nc.vector.tensor_scalar_mul(
                            y_sb[:, m2, :],
                            ps_y[m2],
                            scs_sb[:, e, FT + m2 : FT + m2 + 1],
                        )
                        return
                    nc.scalar.activation(
                        y_sb[:, m2, :],
                        ps_y[m2],
                        mybir.ActivationFunctionType.Identity,
                        bias=(
                            scs_sb[
                                :, e, FT + KT + FT + m2 : FT + KT + FT + m2 + 1
                            ]
                            if with_bias
                            else 0.0
                        ),
                        scale=scs_sb[:, e, FT + m2 : FT + m2 + 1],
                    )

                last = e == EPC - 1
                yt_v = yt[e].rearrange("p (t n) -> p t n", t=KT)
                for tp in range(TP - 1 if last else TP):
                    for m2 in range(KT):
                        mm2(tp, m2)
                if not last:
                    for m2 in range(KT):
                        act_y(m2)

                    def defer(yv=yt_v, ys=y_sb):
                        nc.sync.dma_start(out=yv, in_=ys)

                    deferred_yt[0] = defer
                else:
                    # retire each group on only its 3 last-t-pair matmuls:
                    # the final w2 chunk (the last byte off HBM) gates just
                    # 12 matmuls total, and the acts/yt DMAs stagger behind
                    # it group by group (pieces >=992B/partition, full-rate)
                    for m2 in range(KT):
                        mm2(TP - 1, m2)
                        act_y(m2, on_dve=(m2 % 2 == 1))
                        if m2 == 1:
                            nc.sync.dma_start(
                                out=yt_v[:, 0:2, :], in_=y_sb[:, 0:2, :]
                            )
                        elif m2 == 3:
                            # different queue than the first piece so the
                            # two terminal stores overlap
                            nc.scalar.dma_start(
                                out=yt_v[:, 2:4, :], in_=y_sb[:, 2:4, :]
                            )
    nc.finalize()
    return nc


def get_nc(rep=1, act=None, wa=CAP, wb=CAP, with_bias=True):
    key = (rep, act, wa, wb, with_bias)
    if key not in _CACHE:
        _CACHE[key] = _build_nc(rep, act, wa, wb, with_bias)
    return _CACHE[key]


def _route_np(routes):
    """Numpy replica of the reference's capacity-gated routing."""
    e_map = (routes.astype(np.int64) * E) // B                  # [B, K]
    sel0 = np.zeros((B, E), bool)
    np.put_along_axis(sel0, e_map, True, axis=1)
    sel0_i = sel0.astype(np.int32)
    cum = np.cumsum(sel0_i, axis=0) - sel0_i                    # exclusive cumsum
    selected = sel0 & (cum < CAP)
    slot = cum
    used = selected.sum(axis=1)
    tok_of_slot = np.zeros(E * CAP, np.int32)
    valid = np.zeros(E * CAP, bool)
    b_idx, e_idx = np.nonzero(selected)
    flat = e_idx * CAP + slot[b_idx, e_idx]
    tok_of_slot[flat] = b_idx
    valid[flat] = True
    return tok_of_slot, valid, used, selected, slot


def _assign(routing):
    """Pair experts heavy-with-light across cores.

    Returns (order, WA, WB): order[c] = (expert_A, expert_B) for core c,
    widths WA >= WB are maxima over cores, rounded up to 8.
    """
    selected = routing[3]
    loads = selected.sum(axis=0)
    s = np.argsort(-loads, kind="stable")
    order = [(int(s[i]), int(s[E - 1 - i])) for i in range(N_CORES)]
    wa = max(int(loads[a]) for a, _ in order)
    wb = max(int(loads[b]) for _, b in order)
    wa = min(CAP, max(32, -(-wa // 8) * 8))
    wb = min(CAP, max(32, -(-wb // 8) * 8))
    return order, wa, wb


def _fp8(v):
    return v.astype(NPFP8)


def _hilo(vs):
    """Same-scale hi/lo fp8 split of (already scaled) values."""
    hi = _fp8(vs)
    lo = _fp8(vs - hi.astype(np.float32))
    return hi, lo


def _col_scale(W):
    return WTGT / np.maximum(np.abs(W).max(axis=0), 1e-30)


def _pack_w1(W1e, s1):
    # scaled W1 [D, F] -> [CH1, P, hl*kp*i*256]; row kp*256+i*128+p,
    # f block c*256+fb
    Ws = W1e * s1[None, :]
    hi, lo = _hilo(Ws)
    # [D, F] index: [kp(2), i(2), p(128), c(8), fb(256)]
    def arr(v):
        return v.reshape(KP, 2, P, CH1, M_PER * P).transpose(3, 2, 0, 1, 4)
    a = np.stack([arr(hi), arr(lo)], axis=2)  # [c, p, hl, kp, i, fb]
    return np.ascontiguousarray(a.reshape(CH1, P, -1))


def _pack_w2(W2e, s2):
    # scaled W2 [F, D] -> [CH2, P, hl*tpl*i*512]; row (2c+tpl)*256+i*128+p
    Ws = W2e * s2[None, :]
    hi, lo = _hilo(Ws)
    # [F, D] index: [c(4), tpl(2), i(2), p(128), d(512)]
    def arr(v):
        return v.reshape(CH2, T_PER, 2, P, D).transpose(0, 3, 1, 2, 4)
    a = np.stack([arr(hi), arr(lo)], axis=2)  # [c, p, hl, tpl, i, d]
    return np.ascontiguousarray(a.reshape(CH2, P, -1))


def _pack_xg(x, routing, expert, width):
    """Gather expert tokens -> [P, hl*kp*i*width] fp8 (scaled by SX)."""
    tok_of_slot, valid, used, selected, slot = routing
    sl = slice(expert * CAP, expert * CAP + width)
    xgE = x[tok_of_slot[sl]] * valid[sl, None].astype(np.float32)   # [w, D]
    xs = xgE.T * SX                                                 # [D, w]
    hi, lo = _hilo(xs)
    # [D, w] -> [kp, i, p, w] -> [p, hl, kp, i, w]
    def arr(v):
        return v.reshape(KP, 2, P, width)
    a = np.stack([arr(hi), arr(lo)], axis=1)    # [kp, hl, i?] careful below
    # arr gives [kp, i, p, w]; stack axis=1 -> [kp, hl, i, p, w]
    a = a.transpose(3, 1, 0, 2, 4)              # [p, hl, kp, i, w]
    return np.ascontiguousarray(a.reshape(P, -1))


def _prep_in_maps(x, W1, b1, W2, b2, routing, order, wa, wb, with_bias=True):
    widths = (wa, wb)
    NSC = 2 * (FT + KT)
    in_maps = []
    s1s = [_col_scale(W1[e]) for e in range(E)]
    s2s = [_col_scale(W2[e]) for e in range(E)]
    w1p = {e: _pack_w1(W1[e], s1s[e]) for e in range(E)}
    w2p = {e: _pack_w2(W2[e], s2s[e]) for e in range(E)}
    for c in range(N_CORES):
        es = order[c]
        m = {
            "w1": np.stack([w1p[e] for e in es]),
            "w2": np.stack([w2p[e] for e in es]),
        }
        for s, e in enumerate(es):
            m[f"xg{s}"] = _pack_xg(x, routing, e, widths[s])
        scs = np.zeros((P, EPC, NSC), np.float32)
        for s, e in enumerate(es):
            scs[:, s, :FT] = (1.0 / (SX * s1s[e])).reshape(FT, P).T
            scs[:, s, FT : FT + KT] = (1.0 / s2s[e]).reshape(KT, P).T
            if with_bias:
                scs[:, s, FT + KT : FT + KT + FT] = b1[e].reshape(FT, P).T
                scs[:, s, FT + KT + FT :] = b2[e].reshape(KT, P).T
        m["scs"] = np.ascontiguousarray(scs.reshape(P, -1))
        in_maps.append(m)
    return in_maps


def _erf(v):
    # Abramowitz & Stegun 7.1.26, |err| <= 1.5e-7
    s = np.sign(v)
    a = np.abs(v)
    t = 1.0 / (1.0 + 0.3275911 * a)
    poly = t * (
        0.254829592
        + t * (-0.284496736 + t * (1.421413741 + t * (-1.453152027 + t * 1.061405429)))
    )
    return s * (1.0 - poly * np.exp(-a * a))


def _gelu_exact(v):
    return 0.5 * v * (1.0 + _erf(v / np.sqrt(2.0)))


def kernel(x, W1, b1, W2, b2, Wf1, bf1, Wf2, bf2, routes):
    x = np.asarray(x, np.float32)
    W1 = np.asarray(W1, np.float32)
    b1 = np.asarray(b1, np.float32)
    W2 = np.asarray(W2, np.float32)
    b2 = np.asarray(b2, np.float32)
    Wf1 = np.asarray(Wf1, np.float32)
    bf1 = np.asarray(bf1, np.float32)
    Wf2 = np.asarray(Wf2, np.float32)
    bf2 = np.asarray(bf2, np.float32)
    routes = np.asarray(routes)

    routing = _route_np(routes)
    tok_of_slot, valid, used, selected, slot = routing
    order, wa, wb = _assign(routing)
    with_bias = bool(np.any(b1) or np.any(b2))
    in_maps = _prep_in_maps(x, W1, b1, W2, b2, routing, order, wa, wb, with_bias)

    nc = get_nc(wa=wa, wb=wb, with_bias=with_bias)
    res = run_bass_kernel_spmd(nc, in_maps, core_ids=list(range(N_CORES)))

    # Per-expert outputs [D, width] f32 (garbage in invalid slots; never
    # read there).
    widths = (wa, wb)
    exp_out = [None] * E
    for c in range(N_CORES):
        for s, e in enumerate(order[c]):
            Yc = res.results[c][f"yt{s}"].astype(np.float32)
            exp_out[e] = Yc.reshape(P, KT, widths[s]).transpose(1, 0, 2).reshape(
                D, widths[s]
            )

    # Combine: each token was selected by <= 2 experts; gather its slot
    # outputs and average. Pure host-side gather.
    out = np.zeros((B, D), np.float32)
    b_idx, e_idx = np.nonzero(selected)                         # ordered by token
    s_of = slot[b_idx, e_idx]
    for e in range(E):
        msk = e_idx == e
        out[b_idx[msk]] += exp_out[e][:, s_of[msk]].T
    inv = (1.0 / np.maximum(used, 1)).astype(np.float32)
    out *= inv[:, None]

    # Overflow tokens (used == 0): exact fallback FFN on host.
    ovf = np.nonzero(used == 0)[0]
    if ovf.size:
        xo = x[ovf]
        fb = _gelu_exact(xo @ Wf1 + bf1) @ Wf2 + bf2
        out[ovf] = fb.astype(np.float32)

    return out.astype(np.float32)


# revision 26
# speedup vs baseline: 1.1900x; 1.0092x over previous
"""MoE FFN (capacity-gated routing) on 8 Trainium2 NeuronCores.

Strategy
--------
Expert-parallel: 16 experts / 8 cores = 2 experts per core; host-side
routing/dispatch/combine (full-I/O contract makes the dispatch gather
the sharding step). Experts are sorted by load and paired
heavy-with-light across cores, so the SPMD program uses two static
token widths (WA, WB) = per-slot maxima over cores.

v3: the grouped FFN runs on fp8 (float8e4 = e4m3) with DoubleRow
matmuls (2 contraction rows per partition, 0.5 cycles/row -> 4x bf16
column throughput) while keeping ~bf16 accuracy via an error-split:
every operand is represented as hi + lo fp8 terms sharing one scale
(x, W1, W2 split on the host; h split on-device: ACT writes f32 h,
DVE casts hh=fp8(h) and hl=fp8(h-hh)). Each matmul accumulates the
three significant cross terms (hi*hi + hi*lo + lo*hi) into one PSUM
group; the dropped lo*lo term is O(3e-4) relative. Per-column weight
scales (folded with the global x scale) are applied as per-partition
ScalarE activation scales on PSUM readout, so dequantization is free.
Weight bytes stay 2/element (same DMA as bf16) but PE time drops 25%,
moving the kernel from PE-bound to the HBM roofline. Expert outputs
return as bf16.
"""

import sys

if "/opt/trn_rl_repo" not in sys.path:
    sys.path.append("/opt/trn_rl_repo")

import numpy as np
import ml_dtypes

import concourse.tile as tile
from concourse import bacc, mybir
from concourse.bass_utils import run_bass_kernel_spmd

# Problem shape (hardcoded per contract)
D = 512        # d_model
F = 2048       # d_ff
E = 16         # num experts
B = 2048       # max tokens
CAP = 320      # per-expert capacity = int(1.25 * ceil(B * 2 / E))
N_CORES = 8
EPC = E // N_CORES  # experts per core

P = 128
KT = D // P    # k-tiles over d_model (4)
KP = KT // 2   # DoubleRow k-pair instructions per term (2)
FT = F // P    # tiles over d_ff (16)
TP = FT // 2   # DoubleRow t-pair instructions per term in phase 2 (8)
CH1 = 8        # w1 chunks along f (2 f-tiles each)
CH2 = 8        # w2 chunks along t-pairs (1 t-pair each; the last chunk
               # then gates only 3 matmuls per output group in the tail)
M_PER = FT // CH1   # f-tiles per w1 chunk (2)
T_PER = TP // CH2   # t-pairs per w2 chunk (2)

SX = 16.0      # global x scale into fp8
WTGT = 224.0   # per-column weight scale target (e4m3 max 448/240; stay safe)

FP8 = mybir.dt.float8e4
BF16 = mybir.dt.bfloat16
F32 = mybir.dt.float32
NPFP8 = ml_dtypes.float8_e4m3
NPBF16 = ml_dtypes.bfloat16

DR = mybir.MatmulPerfMode.DoubleRow

_CACHE = {}


def _build_nc(rep=1, act=None, wa=CAP, wb=CAP, with_bias=True):
    """Per-core program: grouped FFN for 2 experts (widths wa >= wb).

    fp8 hi/lo DoubleRow pipeline; see module docstring. All tensors are
    d-major; weights/xg arrive pre-packed in per-partition byte order
    (see _prep_in_maps) so every DMA is [128 x contiguous].
    """
    if act is None:
        act = mybir.ActivationFunctionType.Gelu
    nc = bacc.Bacc(None)
    widths = (wa, wb)
    # xg{s}: [P, hl(2), kp(2), i(2), w] fp8; row (kp*256 + i*128 + p) of
    # scaled x, hi term then lo term.
    xg = [
        nc.declare_dram_parameter(f"xg{s}", [P, 8 * widths[s]], FP8, isOutput=False)
        for s in range(EPC)
    ]
    # w1[e, c]: [P, hl(2), kp(2), i(2), 256] fp8 covering f cols
    # [c*256, (c+1)*256) of scaled W1.
    w1 = nc.declare_dram_parameter(
        "w1", [EPC, CH1, P, 8 * M_PER * P], FP8, isOutput=False
    )
    # w2[e, c]: [P, hl(2), tpl(2), i(2), 512] fp8 covering contraction
    # rows [(2c)*256, (2c+2)*256) of scaled W2.
    w2 = nc.declare_dram_parameter(
        "w2", [EPC, CH2, P, 4 * T_PER * D], FP8, isOutput=False
    )
    # scs: per-partition dequant scales (+ biases when present):
    # [P, s*(2*(FT+KT)) + m]: m<FT: 1/(SX*s1); m<FT+KT: 1/s2;
    # then FT+KT bias entries (b1, b2) in the same order.
    NSC = 2 * (FT + KT)
    scs = nc.declare_dram_parameter("scs", [P, EPC * NSC], F32, isOutput=False)
    yt = [
        nc.declare_dram_parameter(f"yt{s}", [P, KT * widths[s]], BF16, isOutput=True)
    for s in range(EPC)
    ]

    with (
        tile.TileContext(nc) as tc,
        tc.tile_pool(name="consts", bufs=1) as consts,
        tc.tile_pool(name="xgp", bufs=2) as xgp,
        tc.tile_pool(name="w1p", bufs=2 * CH1 + 1) as w1p,
        tc.tile_pool(name="w2p", bufs=2 * CH2 + 1) as w2p,
        tc.tile_pool(name="hp", bufs=2) as hp,
        tc.tile_pool(name="hhp", bufs=2) as hhp,
        tc.tile_pool(name="hlp", bufs=2) as hlp,
        tc.tile_pool(name="yp", bufs=2) as yp,
        tc.tile_pool(name="ps1", bufs=4, space="PSUM") as ps1,
        tc.tile_pool(name="ps2", bufs=4, space="PSUM") as ps2,
    ):
        scs_sb = consts.tile([P, EPC, NSC], F32, name="scs_sb")
        scs_issued = False

        # PE warm-up: dummy matmuls on a zeroed tile keep the PE busy (and
        # the p-state ramp running) while the first xg/w1 chunks stream in.
        zt = consts.tile([P, max(P, wa)], BF16, name="warm_zt")
        nc.vector.memset(zt, 0.0)
        for _w in range(11):
            pw = ps1.tile([P, wa], F32, tag="ps", name="warm_ps")
            nc.tensor.matmul(pw, zt[:, :P], zt[:, :wa], start=True, stop=True)

        # DMA queue split: same-queue transfers serialize (in-order SEQ)
        # but different engine queues' transfers overlap fully, so the
        # 9MB stream is spread over SP / ACT / Pool. ACT and DVE carry
        # heavy compute (gelu + h-split), so they only get transfers
        # scheduled where their pipelines are otherwise idle.
        def issue_w1(e, c, eng=None):
            w1c = w1p.tile([P, 2, KP, 2, M_PER * P], FP8, tag="w1c", name="w1c")
            (eng or nc.sync).dma_start(
                out=w1c,
                in_=w1[e, c].rearrange(
                    "p (hl kp i f) -> p hl kp i f", hl=2, kp=KP, i=2
                ),
            )
            return w1c

        def issue_xg(e, eng=None):
            nb = widths[e]
            xg_sb = xgp.tile([P, 2, KP, 2, nb], FP8, name="xg_sb")
            (eng or nc.sync).dma_start(
                out=xg_sb,
                in_=xg[e].rearrange("p (hl kp i n) -> p hl kp i n", hl=2, kp=KP, i=2),
            )
            return xg_sb

        prefetched = {}
        deferred_yt = [None]

        for _ in range(rep):
            for e in range(EPC):
                nb = widths[e]
                if e in prefetched:
                    xg_sb, w1c0 = prefetched.pop(e)
                else:
                    # both first-matmul inputs ride SP back-to-back (ACT's
                    # queue head holds the 1.3us act-table load and Pool's
                    # SWDGE generation is ~1us). The hi half of xg goes
                    # first: each psum group's first four matmuls use only
                    # x-hi, so the PE starts before the lo half lands.
                    xg_sb = xgp.tile([P, 2, KP, 2, widths[e]], FP8, name="xg_sb")
                    xg_v = xg[e].rearrange(
                        "p (hl kp i n) -> p hl kp i n", hl=2, kp=KP, i=2
                    )
                    nc.sync.dma_start(
                        out=xg_sb[:, 0:1, :, :, :], in_=xg_v[:, 0:1, :, :, :]
                    )
                    w1c0 = issue_w1(e, 0)
                    nc.sync.dma_start(
                        out=xg_sb[:, 1:2, :, :, :], in_=xg_v[:, 1:2, :, :, :]
                    )
                w1cs = [w1c0]
                h_sb = hp.tile([P, FT, nb], F32, name="h_sb")
                hh_sb = hhp.tile([P, FT, nb], FP8, name="hh_sb")
                hl_sb = hlp.tile([P, FT, nb], FP8, name="hl_sb")

                # phase 1: h = gelu((x @ W1) * scale1); 3 DR terms per psum
                # group; w1 chunked along f so matmuls start after one
                # 256KB chunk lands. The scs DMA and the previous expert's
                # deferred yt store slot into the chunk stream so small
                # transfers never cluster at the head of the HWDGE queue.
                for c in range(CH1):
                    if c > 0:
                        # expert 0's odd chunks ride the Pool queue (idle
                        # until w2 loads begin) so phase 1 is never
                        # chunk-starved; expert 1's stream has slack on SP
                        eng = nc.gpsimd if (e == 0 and c % 2 == 1) else None
                        w1cs.append(issue_w1(e, c, eng=eng))
                    if c == 3 and deferred_yt[0] is not None:
                        deferred_yt[0]()
                        deferred_yt[0] = None
                    w1c = w1cs[c]
                    for ml in range(M_PER):
                        m = c * M_PER + ml
                        ps = ps1.tile([P, nb], F32, name="ps")
                        first = True
                        for whl, xhl in ((0, 0), (1, 0), (0, 1)):
                            for kp in range(KP):
                                nc.tensor.matmul(
                                    ps,
                                    w1c[:, whl, kp, :, ml * P : (ml + 1) * P],
                                    xg_sb[:, xhl, kp, :, :],
                                    start=first,
                                    stop=(whl == 0 and xhl == 1 and kp == KP - 1),
                                    perf_mode=DR,
                                )
                                first = False
                        if not scs_issued:
                            # traced before the first ACT (deps follow trace
                            # order) but issued via the Pool-engine SWDGE
                            # queue so it doesn't take an HWDGE slot from
                            # the weight-chunk stream
                            nc.gpsimd.dma_start(
                                out=scs_sb,
                                in_=scs.rearrange("p (e t) -> p e t", e=EPC),
                            )
                            scs_issued = True
                        nc.scalar.activation(
                            h_sb[:, m, :],
                            ps,
                            act,
                            bias=(
                                scs_sb[:, e, FT + KT + m : FT + KT + m + 1]
                                if with_bias
                                else 0.0
                            ),
                            scale=scs_sb[:, e, m : m + 1],
                        )
                        nc.vector.tensor_copy(out=hh_sb[:, m, :], in_=h_sb[:, m, :])
                        nc.vector.tensor_sub(
                            hl_sb[:, m, :], h_sb[:, m, :], hh_sb[:, m, :]
                        )

                # phase 2: y = (h @ W2) * scale2; tp-outer so each w2 chunk
                # is consumed on arrival; 4 output groups accumulate in
                # parallel across the 4 ps2 banks
                ps_y = [
                    ps2.tile([P, nb], F32, tag="psy", name=f"psy{m2}")
                    for m2 in range(KT)
                ]
                w2cs = []
                for c in range(CH2):
                    w2c = w2p.tile([P, 2, T_PER, 2, D], FP8, tag="w2c", name="w2c")
                    nc.gpsimd.dma_start(
                        out=w2c,
                        in_=w2[e, c].rearrange(
                            "p (hl tpl i d) -> p hl tpl i d", hl=2, tpl=T_PER, i=2
                        ),
                    )
                    w2cs.append(w2c)
                    if c == 3 and e + 1 < EPC:
                        # prefetch the next expert's first-matmul inputs so
                        # its phase 1 starts as soon as this phase 2 ends
                        nw1c0 = issue_w1(e + 1, 0)
                        nxg = issue_xg(e + 1, eng=nc.gpsimd)
                        prefetched[e + 1] = (nxg, nw1c0)

                def mm2(tp, m2):
                    c, tpl = divmod(tp, T_PER)
                    first = True
                    for whl, hsrc in ((0, hh_sb), (1, hh_sb), (0, hl_sb)):
                        nc.tensor.matmul(
                            ps_y[m2],
                            w2cs[c][:, whl, tpl, :, m2 * P : (m2 + 1) * P],
                            hsrc[:, 2 * tp : 2 * tp + 2, :],
                            start=(tp == 0 and first),
                            stop=(tp == TP - 1 and whl == 0 and hsrc is hl_sb),
                            perf_mode=DR,
                        )
                        first = False

                y_sb = yp.tile([P, KT, nb], BF16, name="y_sb")

                def act_y(m2, alt_eng=None):
                    if alt_eng is not None and not with_bias:
                        # spread the final dequant readouts across ACT /
                        # DVE / Pool so the last groups retire in parallel
                        alt_eng.tensor_scalar_mul(
                            y_sb[:, m2, :],
                            ps_y[m2],
                            scs_sb[:, e, FT + m2 : FT + m2 + 1],
                        )
                        return
                    nc.scalar.activation(
                        y_sb[:, m2, :],
                        ps_y[m2],
                        mybir.ActivationFunctionType.Identity,
                        bias=(
                            scs_sb[
                                :, e, FT + KT + FT + m2 : FT + KT + FT + m2 + 1
                            ]
                            if with_bias
                            else 0.0
                        ),
                        scale=scs_sb[:, e, FT + m2 : FT + m2 + 1],
                    )

                last = e == EPC - 1
                yt_v = yt[e].rearrange("p (t n) -> p t n", t=KT)
                for tp in range(TP - 1 if last else TP):
                    for m2 in range(KT):
                        mm2(tp, m2)
                if not last:
                    for m2 in range(KT):
                        act_y(m2)

                    def defer(yv=yt_v, ys=y_sb):
                        nc.sync.dma_start(out=yv, in_=ys)

                    deferred_yt[0] = defer
                else:
                    # retire each group on only its 3 last-t-pair matmuls:
                    # the final w2 chunk (the last byte off HBM) gates just
                    # 12 matmuls total, and the acts/yt DMAs stagger behind
                    # it group by group (pieces >=992B/partition, full-rate)
                    retire_eng = (None, nc.vector, nc.gpsimd, None)
                    for m2 in range(KT):
                        mm2(TP - 1, m2)
                        act_y(m2, alt_eng=retire_eng[m2])
                        if m2 == 1:
                            nc.sync.dma_start(
                                out=yt_v[:, 0:2, :], in_=y_sb[:, 0:2, :]
                            )
                        elif m2 == 3:
                            # different queue than the first piece so the
                            # two terminal stores overlap
                            nc.scalar.dma_start(
                                out=yt_v[:, 2:4, :], in_=y_sb[:, 2:4, :]
                            )
    nc.finalize()
    return nc


def get_nc(rep=1, act=None, wa=CAP, wb=CAP, with_bias=True):
    key = (rep, act, wa, wb, with_bias)
    if key not in _CACHE:
        _CACHE[key] = _build_nc(rep, act, wa, wb, with_bias)
    return _CACHE[key]


def _route_np(routes):
    """Numpy replica of the reference's capacity-gated routing."""
    e_map = (routes.astype(np.int64) * E) // B                  # [B, K]
    sel0 = np.zeros((B, E), bool)
    np.put_along_axis(sel0, e_map, True, axis=1)
    sel0_i = sel0.astype(np.int32)
    cum = np.cumsum(sel0_i, axis=0) - sel0_i                    # exclusive cumsum
    selected = sel0 & (cum < CAP)
    slot = cum
    used = selected.sum(axis=1)
    tok_of_slot = np.zeros(E * CAP, np.int32)
    valid = np.zeros(E * CAP, bool)
    b_idx, e_idx = np.nonzero(selected)
    flat = e_idx * CAP + slot[b_idx, e_idx]
    tok_of_slot[flat] = b_idx
    valid[flat] = True
    return tok_of_slot, valid, used, selected, slot


def _assign(routing):
    """Pair experts heavy-with-light across cores.

    Returns (order, WA, WB): order[c] = (expert_A, expert_B) for core c,
    widths WA >= WB are maxima over cores, rounded up to 8.
    """
    selected = routing[3]
    loads = selected.sum(axis=0)
    s = np.argsort(-loads, kind="stable")
    order = [(int(s[i]), int(s[E - 1 - i])) for i in range(N_CORES)]
    wa = max(int(loads[a]) for a, _ in order)
    wb = max(int(loads[b]) for _, b in order)
    wa = min(CAP, max(32, wa))
    wb = min(CAP, max(32, wb))
    return order, wa, wb


def _fp8(v):
    return v.astype(NPFP8)


def _hilo(vs):
    """Same-scale hi/lo fp8 split of (already scaled) values."""
    hi = _fp8(vs)
    lo = _fp8(vs - hi.astype(np.float32))
    return hi, lo


def _col_scale(W):
    return WTGT / np.maximum(np.abs(W).max(axis=0), 1e-30)


def _pack_w1(W1e, s1):
    # scaled W1 [D, F] -> [CH1, P, hl*kp*i*256]; row kp*256+i*128+p,
    # f block c*256+fb
    Ws = W1e * s1[None, :]
    hi, lo = _hilo(Ws)
    # [D, F] index: [kp(2), i(2), p(128), c(8), fb(256)]
    def arr(v):
        return v.reshape(KP, 2, P, CH1, M_PER * P).transpose(3, 2, 0, 1, 4)
    a = np.stack([arr(hi), arr(lo)], axis=2)  # [c, p, hl, kp, i, fb]
    return np.ascontiguousarray(a.reshape(CH1, P, -1))


def _pack_w2(W2e, s2):
    # scaled W2 [F, D] -> [CH2, P, hl*tpl*i*512]; row (2c+tpl)*256+i*128+p
    Ws = W2e * s2[None, :]
    hi, lo = _hilo(Ws)
    # [F, D] index: [c(4), tpl(2), i(2), p(128), d(512)]
    def arr(v):
        return v.reshape(CH2, T_PER, 2, P, D).transpose(0, 3, 1, 2, 4)
    a = np.stack([arr(hi), arr(lo)], axis=2)  # [c, p, hl, tpl, i, d]
    return np.ascontiguousarray(a.reshape(CH2, P, -1))


def _pack_xg(x, routing, expert, width):
    """Gather expert tokens -> [P, hl*kp*i*width] fp8 (scaled by SX)."""
    tok_of_slot, valid, used, selected, slot = routing
    sl = slice(expert * CAP, expert * CAP + width)
    xgE = x[tok_of_slot[sl]] * valid[sl, None].astype(np.float32)   # [w, D]
    xs = xgE.T * SX                                                 # [D, w]
    hi, lo = _hilo(xs)
    # [D, w] -> [kp, i, p, w] -> [p, hl, kp, i, w]
    def arr(v):
        return v.reshape(KP, 2, P, width)
    a = np.stack([arr(hi), arr(lo)], axis=1)    # [kp, hl, i?] careful below
    # arr gives [kp, i, p, w]; stack axis=1 -> [kp, hl, i, p, w]
    a = a.transpose(3, 1, 0, 2, 4)              # [p, hl, kp, i, w]
    return np.ascontiguousarray(a.reshape(P, -1))


def _prep_in_maps(x, W1, b1, W2, b2, routing, order, wa, wb, with_bias=True):
    widths = (wa, wb)
    NSC = 2 * (FT + KT)
    in_maps = []
    s1s = [_col_scale(W1[e]) for e in range(E)]
    s2s = [_col_scale(W2[e]) for e in range(E)]
    w1p = {e: _pack_w1(W1[e], s1s[e]) for e in range(E)}
    w2p = {e: _pack_w2(W2[e], s2s[e]) for e in range(E)}
    for c in range(N_CORES):
        es = order[c]
        m = {
            "w1": np.stack([w1p[e] for e in es]),
            "w2": np.stack([w2p[e] for e in es]),
        }
        for s, e in enumerate(es):
            m[f"xg{s}"] = _pack_xg(x, routing, e, widths[s])
        scs = np.zeros((P, EPC, NSC), np.float32)
        for s, e in enumerate(es):
            scs[:, s, :FT] = (1.0 / (SX * s1s[e])).reshape(FT, P).T
            scs[:, s, FT : FT + KT] = (1.0 / s2s[e]).reshape(KT, P).T
            if with_bias:
                scs[:, s, FT + KT : FT + KT + FT] = b1[e].reshape(FT, P).T
                scs[:, s, FT + KT + FT :] = b2[e].reshape(KT, P).T
        m["scs"] = np.ascontiguousarray(scs.reshape(P, -1))
        in_maps.append(m)
    return in_maps


def _erf(v):
    # Abramowitz & Stegun 7.1.26, |err| <= 1.5e-7
    s = np.sign(v)
    a = np.abs(v)
    t = 1.0 / (1.0 + 0.3275911 * a)
    poly = t * (
        0.254829592
        + t * (-0.284496736 + t * (1.421413741 + t * (-1.453152027 + t * 1.061405429)))
    )
    return s * (1.0 - poly * np.exp(-a * a))


def _gelu_exact(v):
    return 0.5 * v * (1.0 + _erf(v / np.sqrt(2.0)))


def kernel(x, W1, b1, W2, b2, Wf1, bf1, Wf2, bf2, routes):
    x = np.asarray(x, np.float32)
    W1 = np.asarray(W1, np.float32)
    b1 = np.asarray(b1, np.float32)
    W2 = np.asarray(W2, np.float32)
    b2 = np.asarray(b2, np.float32)
    Wf1 = np.asarray(Wf1, np.float32)
    bf1 = np.asarray(bf1, np.float32)
    Wf2 = np.asarray(Wf2, np.float32)
    bf2 = np.asarray(bf2, np.float32)
    routes = np.asarray(routes)

    routing = _route_np(routes)
    tok_of_slot, valid, used, selected, slot = routing
    order, wa, wb = _assign(routing)
    with_bias = bool(np.any(b1) or np.any(b2))
    in_maps = _prep_in_maps(x, W1, b1, W2, b2, routing, order, wa, wb, with_bias)

    nc = get_nc(wa=wa, wb=wb, with_bias=with_bias)
    res = run_bass_kernel_spmd(nc, in_maps, core_ids=list(range(N_CORES)))

    # Per-expert outputs [D, width] f32 (garbage in invalid slots; never
    # read there).
    widths = (wa, wb)
    exp_out = [None] * E
    for c in range(N_CORES):
        for s, e in enumerate(order[c]):
            Yc = res.results[c][f"yt{s}"].astype(np.float32)
            exp_out[e] = Yc.reshape(P, KT, widths[s]).transpose(1, 0, 2).reshape(
                D, widths[s]
            )

    # Combine: each token was selected by <= 2 experts; gather its slot
    # outputs and average. Pure host-side gather.
    out = np.zeros((B, D), np.float32)
    b_idx, e_idx = np.nonzero(selected)                         # ordered by token
    s_of = slot[b_idx, e_idx]
    for e in range(E):
        msk = e_idx == e
        out[b_idx[msk]] += exp_out[e][:, s_of[msk]].T
    inv = (1.0 / np.maximum(used, 1)).astype(np.float32)
    out *= inv[:, None]

    # Overflow tokens (used == 0): exact fallback FFN on host.
    ovf = np.nonzero(used == 0)[0]
    if ovf.size:
        xo = x[ovf]
        fb = _gelu_exact(xo @ Wf1 + bf1) @ Wf2 + bf2
        out[ovf] = fb.astype(np.float32)

    return out.astype(np.float32)
